# revision 1
# baseline (speedup 1.0000x reference)
"""Grouped-Query Attention kernel for Trainium2, 8-core SPMD.

Problem (full shapes): B=2, S=2048, D=2048, H=32 q-heads, KV=8 kv-heads,
DK=64, REP=4.

Sharding: 16 (batch, kv-group) units over 8 cores -> each core owns one
batch b and 2 adjacent kv-groups (8 query heads, 512 q-cols / 128 kv-cols).
Each core computes its heads' attention output and a partial output
projection against its 512-row slice of Wo; the host sums the 4 partials
per batch and adds bo.

Everything on-chip lives in "transposed" space (feature dim on SBUF
partitions): the host passes x pre-transposed (xT = x[b].T) so no on-chip
transposition of activations is needed, scores are computed directly as
P^T = exp((K^T)^T-style matmuls with t on PSUM partitions, and the final
output is produced as outT = Wo_slice^T @ attn_out^T, un-transposed on the
host.
"""

import os
from contextlib import ExitStack

import numpy as np

import concourse.bass as bass
import concourse.tile as tile
from concourse import bacc
from concourse import mybir
from concourse.masks import make_identity

F32 = mybir.dt.float32
F16 = mybir.dt.float16

# Full-problem constants (hardcoded per contest contract).
B = 2
S = 2048
D = 2048
H = 32
KV = 8
DK = 64
REP = H // KV          # 4
NCORES = 8

GPC = (KV * B) // NCORES      # kv-groups per core = 2
QC = GPC * REP * DK           # local q cols = 512
KC = GPC * DK                 # local k cols = 128
HL = GPC * REP                # local heads = 8
SB = 512                      # s-block size
NB = S // SB                  # 4 blocks
NKD = D // 128                # 16 contraction chunks for projections
NQT = QC // 128               # 4 q-col tiles
NPR = QC // 128               # 4 head-pair tiles (rhs chunks for out proj)
NOT = D // 128                # 16 out-col tiles
TPB = SB // 128               # 4 t-chunks per s-block

NEG = -1.0e30


def build_gqa_nc():
    nc = bacc.Bacc("TRN2", target_bir_lowering=False, debug=False)

    xT = nc.dram_tensor("xT", [D, S], F16, kind="ExternalInput").ap()
    wq = nc.dram_tensor("wq", [D, QC], F16, kind="ExternalInput").ap()
    wk = nc.dram_tensor("wk", [D, KC], F16, kind="ExternalInput").ap()
    wv = nc.dram_tensor("wv", [D, KC], F16, kind="ExternalInput").ap()
    wo = nc.dram_tensor("wo", [QC, D], F16, kind="ExternalInput").ap()
    bq = nc.dram_tensor("bq", [QC], F32, kind="ExternalInput").ap()
    bk = nc.dram_tensor("bk", [KC], F32, kind="ExternalInput").ap()
    bv = nc.dram_tensor("bv", [KC], F32, kind="ExternalInput").ap()
    outT = nc.dram_tensor("outT", [D, S], F32, kind="ExternalOutput").ap()

    with tile.TileContext(nc) as tc, ExitStack() as ctx:
        singles = ctx.enter_context(tc.tile_pool(name="singles", bufs=1))
        wpool = ctx.enter_context(tc.tile_pool(name="wpool", bufs=1))
        xtp = ctx.enter_context(tc.tile_pool(name="xtp", bufs=2))
        qtp = ctx.enter_context(tc.tile_pool(name="qtp", bufs=2))
        vtp = ctx.enter_context(tc.tile_pool(name="vtp", bufs=2))
        ptp = ctx.enter_context(tc.tile_pool(name="ptp", bufs=8))
        atp = ctx.enter_context(tc.tile_pool(name="atp", bufs=2))
        otp = ctx.enter_context(tc.tile_pool(name="otp", bufs=3))
        smp = ctx.enter_context(tc.tile_pool(name="smp", bufs=4))

        pp_pj = ctx.enter_context(tc.tile_pool(name="pp_pj", bufs=2, space="PSUM"))
        pp_tr = ctx.enter_context(tc.tile_pool(name="pp_tr", bufs=1, space="PSUM"))
        pp_sc = ctx.enter_context(tc.tile_pool(name="pp_sc", bufs=2, space="PSUM"))
        pp_av = ctx.enter_context(tc.tile_pool(name="pp_av", bufs=2, space="PSUM"))
        pp_bc = ctx.enter_context(tc.tile_pool(name="pp_bc", bufs=1, space="PSUM"))

        # ---- constants ----
        ident = singles.tile([128, 128], F16, name="ident", tag="ident")
        make_identity(nc, ident)

        # mask0[r, c] = 0 where c >= r else NEG (applied to diagonal tiles;
        # a diagonal tile at relative offset k uses mask0[:, : SB - 128 k]
        # against psum cols [128 k :]).
        mask0 = singles.tile([128, SB], F32, name="mask0", tag="mask0")
        nc.gpsimd.memset(mask0, 0.0)
        nc.gpsimd.affine_select(
            out=mask0,
            in_=mask0,
            compare_op=mybir.AluOpType.is_ge,
            fill=NEG,
            base=0,
            pattern=[[1, SB]],
            channel_multiplier=-1,
        )

        ones1 = singles.tile([1, DK], F16, name="ones1", tag="ones1")
        nc.vector.memset(ones1, 1.0)

        sbq = singles.tile([128, NQT], F32, name="sbq", tag="sbq")
        nc.sync.dma_start(out=sbq, in_=bq.rearrange("(t p) -> p t", p=128))
        sbk = singles.tile([128, 1], F32, name="sbk", tag="sbk")
        nc.sync.dma_start(out=sbk, in_=bk.rearrange("(t p) -> p t", p=128))
        sbv = singles.tile([128, 1], F32, name="sbv", tag="sbv")
        nc.sync.dma_start(out=sbv, in_=bv.rearrange("(t p) -> p t", p=128))

        # ---- persistent weights ----
        wq_t = []
        for kd in range(NKD):
            t = wpool.tile([128, QC], F16, name=f"wq{kd}", tag=f"wq{kd}")
            nc.sync.dma_start(out=t, in_=wq[kd * 128:(kd + 1) * 128, :])
            wq_t.append(t)
        wk_t = []
        wv_t = []
        for kd in range(NKD):
            t = wpool.tile([128, KC], F16, name=f"wk{kd}", tag=f"wk{kd}")
            nc.sync.dma_start(out=t, in_=wk[kd * 128:(kd + 1) * 128, :])
            wk_t.append(t)
            t2 = wpool.tile([128, KC], F16, name=f"wv{kd}", tag=f"wv{kd}")
            nc.sync.dma_start(out=t2, in_=wv[kd * 128:(kd + 1) * 128, :])
            wv_t.append(t2)
        wo_t = []
        for pr in range(NPR):
            t = wpool.tile([128, D], F16, name=f"wo{pr}", tag=f"wo{pr}")
            nc.sync.dma_start(out=t, in_=wo[pr * 128:(pr + 1) * 128, :])
            wo_t.append(t)

        # ---- persistent K^T and V_aug ----
        kT_all = wpool.tile([128, S], F16, name="kT_all", tag="kT_all")
        # vaug[g][j][:, tt, 0:64] = V rows for t-chunk (j*TPB+tt), group g;
        # col 64 = ones (folds the softmax denominator into the AV matmul).
        vaug = [[None] * NB for _ in range(GPC)]
        for g in range(GPC):
            for j in range(NB):
                t = wpool.tile(
                    [128, TPB, DK + 1], F16,
                    name=f"vaug{g}_{j}", tag=f"vaug{g}_{j}",
                )
                nc.vector.memset(t[:, :, DK:DK + 1], 1.0)
                vaug[g][j] = t

        # ---- main loop over s-blocks ----
        for j in range(NB):
            s0 = j * SB

            # xT tiles for this block: [128 d, SB s] each.
            xt = []
            for kd in range(NKD):
                t = xtp.tile([128, SB], F16, name=f"xt{kd}", tag=f"xt{kd}")
                nc.sync.dma_start(
                    out=t, in_=xT[kd * 128:(kd + 1) * 128, s0:s0 + SB]
                )
                xt.append(t)

            # Q^T projection: qT[qt] = (Wq_chunk^T @ xT_chunk summed) + bq
            qT = []
            for qt in range(NQT):
                ps = pp_pj.tile([128, SB], F32, name="ps_q", tag="pj")
                for kd in range(NKD):
                    nc.tensor.matmul(
                        out=ps,
                        lhsT=wq_t[kd][:, qt * 128:(qt + 1) * 128],
                        rhs=xt[kd],
                        start=(kd == 0),
                        stop=(kd == NKD - 1),
                    )
                t = qtp.tile([128, SB], F16, name=f"qT{qt}", tag=f"qT{qt}")
                nc.scalar.activation(
                    out=t, in_=ps,
                    func=mybir.ActivationFunctionType.Identity,
                    bias=sbq[:, qt:qt + 1],
                )
                qT.append(t)

            # K^T projection -> persistent kT_all columns [s0:s0+SB].
            ps_k = pp_pj.tile([128, SB], F32, name="ps_k", tag="pj")
            for kd in range(NKD):
                nc.tensor.matmul(
                    out=ps_k, lhsT=wk_t[kd], rhs=xt[kd],
                    start=(kd == 0), stop=(kd == NKD - 1),
                )
            nc.scalar.activation(
                out=kT_all[:, s0:s0 + SB], in_=ps_k,
                func=mybir.ActivationFunctionType.Identity,
                bias=sbk,
            )

            # V^T projection (transient), then PE-transpose into vaug.
            ps_v = pp_pj.tile([128, SB], F32, name="ps_v", tag="pj")
            for kd in range(NKD):
                nc.tensor.matmul(
                    out=ps_v, lhsT=wv_t[kd], rhs=xt[kd],
                    start=(kd == 0), stop=(kd == NKD - 1),
                )
            vT = vtp.tile([128, SB], F16, name="vT", tag="vT")
            nc.scalar.activation(
                out=vT, in_=ps_v,
                func=mybir.ActivationFunctionType.Identity,
                bias=sbv,
            )
            for tt in range(TPB):
                for g in range(GPC):
                    ps_t = pp_tr.tile([128, DK], F16, name="ps_t", tag="tr")
                    nc.tensor.transpose(
                        out=ps_t,
                        in_=vT[g * DK:(g + 1) * DK, tt * 128:(tt + 1) * 128],
                        identity=ident[g * DK:(g + 1) * DK, g * DK:(g + 1) * DK],
                    )
                    nc.vector.tensor_copy(
                        out=vaug[g][j][:, tt, 0:DK], in_=ps_t
                    )

            # Attention per local head.
            nti = TPB * (j + 1)  # t-chunks needed for this block
            # Host permutes Wq cols / Wo rows so q-tile m holds head m
            # (group 0) in partitions 0:64 and head 4+m (group 1) in
            # partitions 64:128 -- q rows then share the base partition of
            # the head's K^T rows, as the PE quadrant tiling requires.
            apairs = []
            for hl in range(HL):
                g = hl // REP
                qrow = g * DK
                qtile = qT[hl % REP]
                ps_av = pp_av.tile([DK + 1, SB], F32, name="ps_av", tag="av")
                for ti in range(nti):
                    krel = ti - TPB * j
                    c0 = 128 * krel if krel > 0 else 0
                    ps_p = pp_sc.tile([128, SB], F32, name="ps_p", tag="sc")
                    nc.tensor.matmul(
                        out=ps_p[:, c0:SB],
                        lhsT=kT_all[g * DK:(g + 1) * DK,
                                    ti * 128:(ti + 1) * 128],
                        rhs=qtile[qrow:qrow + DK, c0:SB],
                        start=True, stop=True,
                    )
                    if krel >= 0:
                        # diagonal tile: additive causal mask
                        nc.vector.tensor_add(
                            out=ps_p[:, c0:SB],
                            in0=ps_p[:, c0:SB],
                            in1=mask0[:, 0:SB - c0],
                        )
                    pt = ptp.tile([128, SB], F16, name="pt", tag="pt")
                    nc.scalar.activation(
                        out=pt[:, c0:SB], in_=ps_p[:, c0:SB],
                        func=mybir.ActivationFunctionType.Exp,
                        scale=0.125,
                    )
                    nc.tensor.matmul(
                        out=ps_av[:, c0:SB],
                        lhsT=vaug[g][ti // TPB][:, ti % TPB, :],
                        rhs=pt[:, c0:SB],
                        start=(ti == 0),
                        stop=(ti == nti - 1),
                    )
                # normalize: out_h^T = ps_av[0:DK] * (1 / ps_av[DK])
                r1 = smp.tile([1, SB], F32, name="r1", tag="r1")
                nc.vector.reciprocal(out=r1, in_=ps_av[DK:DK + 1, :])
                r1h = smp.tile([1, SB], F16, name="r1h", tag="r1h")
                nc.gpsimd.tensor_copy(out=r1h, in_=r1)
                # replicate r1 across 64 partitions via a K=1 PE matmul
                bc = pp_bc.tile([DK, SB], F32, name="bc", tag="bc")
                nc.tensor.matmul(
                    out=bc, lhsT=ones1, rhs=r1h, start=True, stop=True
                )
                pr = hl % REP
                half = hl // REP
                if half == 0:
                    apair = atp.tile(
                        [128, SB], F16, name=f"ap{pr}", tag=f"ap{pr}"
                    )
                    apairs.append(apair)
                av_s = smp.tile([DK, SB], F32, name="av_s", tag="av_s")
                nc.vector.tensor_copy(out=av_s, in_=ps_av[0:DK, :])
                nc.vector.tensor_mul(
                    out=apairs[pr][half * DK:(half + 1) * DK, :],
                    in0=av_s,
                    in1=bc,
                )

            # Output projection: outT[:, s0:s0+SB] partial.
            for ot in range(NOT):
                ps_o = pp_pj.tile([128, SB], F32, name="ps_o", tag="pj")
                for pr in range(NPR):
                    nc.tensor.matmul(
                        out=ps_o,
                        lhsT=wo_t[pr][:, ot * 128:(ot + 1) * 128],
                        rhs=apairs[pr],
                        start=(pr == 0),
                        stop=(pr == NPR - 1),
                    )
                osb = otp.tile([128, SB], F32, name="osb", tag="osb")
                nc.vector.tensor_copy(out=osb, in_=ps_o)
                nc.sync.dma_start(
                    out=outT[ot * 128:(ot + 1) * 128, s0:s0 + SB], in_=osb
                )

    nc.compile()
    return nc


def make_in_maps(x, Wq, bq, Wk, bk, Wv, bv, Wo, bo):
    x = np.asarray(x, dtype=np.float32)
    Wq = np.asarray(Wq, dtype=np.float32)
    Wk = np.asarray(Wk, dtype=np.float32)
    Wv = np.asarray(Wv, dtype=np.float32)
    Wo = np.asarray(Wo, dtype=np.float32)
    bq = np.asarray(bq, dtype=np.float32)
    bk = np.asarray(bk, dtype=np.float32)
    bv = np.asarray(bv, dtype=np.float32)
    # Local-head layout permutation: q-tile m = [head m (g0) | head 4+m (g1)]
    perm = [0, REP, 1, REP + 1, 2, REP + 2, 3, REP + 3][:HL]
    in_maps = []
    for c in range(NCORES):
        b = c // (NCORES // B)
        gp = c % (NCORES // B)
        q0 = gp * QC
        k0 = gp * KC
        qcols = np.concatenate(
            [np.arange(q0 + hl * DK, q0 + (hl + 1) * DK) for hl in perm]
        )
        in_maps.append({
            "xT": np.ascontiguousarray(x[b].T.astype(np.float16)),
            "wq": np.ascontiguousarray(Wq[:, qcols].astype(np.float16)),
            "wk": np.ascontiguousarray(Wk[:, k0:k0 + KC].astype(np.float16)),
            "wv": np.ascontiguousarray(Wv[:, k0:k0 + KC].astype(np.float16)),
            "wo": np.ascontiguousarray(Wo[qcols, :].astype(np.float16)),
            "bq": np.ascontiguousarray(bq[qcols]),
            "bk": np.ascontiguousarray(bk[k0:k0 + KC]),
            "bv": np.ascontiguousarray(bv[k0:k0 + KC]),
        })
    return in_maps


def assemble_output(results, bo):
    bo = np.asarray(bo, dtype=np.float32)
    out = np.zeros((B, S, D), dtype=np.float32)
    for c in range(NCORES):
        b = c // (NCORES // B)
        out[b] += results[c]["outT"].T
    out += bo
    return out


_NC_CACHE = None


def kernel(x, Wq, bq, Wk, bk, Wv, bv, Wo, bo):
    global _NC_CACHE
    from concourse.bass_utils import run_bass_kernel_spmd

    if _NC_CACHE is None:
        _NC_CACHE = build_gqa_nc()
    nc = _NC_CACHE
    in_maps = make_in_maps(x, Wq, bq, Wk, bk, Wv, bv, Wo, bo)
    res = run_bass_kernel_spmd(nc, in_maps, list(range(NCORES))).results
    return assemble_output(res, bo)



# revision 2
# speedup vs baseline: 1.0341x; 1.0341x over previous
"""Grouped-Query Attention kernel for Trainium2, 8-core SPMD. v2.

Problem (full shapes): B=2, S=2048, D=2048, H=32 q-heads, KV=8 kv-heads,
DK=64, REP=4.

Sharding: 16 (batch, kv-group) units over 8 cores -> each core owns one
batch b and 2 adjacent kv-groups (8 query heads, 512 q-cols / 128 kv-cols).
Each core computes its heads' attention output and a partial output
projection against its 512-row slice of Wo; the host sums the 4 partials
per batch and adds bo.

v2 restructuring vs v1 (all engine placement driven by the cost model):
- Causal mask applied via an extra PE matmul (identity x f16 mask row)
  accumulated into the score PSUM group - no vector-engine mask work.
- AV matmul flipped: out[s, dk+1] = P^T.T @ V_aug with the P tile as
  stationary (N=65 per chunk instead of 512) - halves AV PE cycles and
  puts the softmax denominator on the partition axis, so normalization
  is one reciprocal + one tensor_scalar_mul on DVE.
- Attention output transposed back to [hd, s] with SBUF->SBUF DMA
  transposes (14 ns per 16x128 tile) instead of PE transposes.
- V projected directly in [t, kv] layout (x chunk as stationary), bias
  folded in via a K=1 ones matmul - no V transposes.
- QKV bias adds on DVE (tensor_scalar_add), Act engine runs exps only.
- Software-pipelined emission: out-proj of block j-1 and QKV proj of
  block j+1 are interleaved into the Act-bound attention phase of
  block j to keep the PE stream fed.
- f16 output partials (halves output DMA; host sums in f32).
"""

import os
from contextlib import ExitStack

import numpy as np

import concourse.bass as bass
import concourse.tile as tile
from concourse import bacc
from concourse import mybir
from concourse.masks import make_identity

F32 = mybir.dt.float32
F16 = mybir.dt.float16

# Full-problem constants (hardcoded per contest contract).
B = 2
S = 2048
D = 2048
H = 32
KV = 8
DK = 64
REP = H // KV          # 4
NCORES = 8

GPC = (KV * B) // NCORES      # kv-groups per core = 2
QC = GPC * REP * DK           # local q cols = 512
KC = GPC * DK                 # local k cols = 128
HL = GPC * REP                # local heads = 8
SB = 512                      # s-block size
NB = S // SB                  # 4 blocks
NKD = D // 128                # 16 contraction chunks for projections
NQT = QC // 128               # 4 q-col tiles
NPR = QC // 128               # 4 head-pair tiles (rhs chunks for out proj)
NOT = D // 128                # 16 out-col tiles
TPB = SB // 128               # 4 t-chunks per s-block

MASK_NEG = -60000.0           # f16-representable; exp(0.125*(s-60000)) == 0


def build_gqa_nc():
    nc = bacc.Bacc("TRN2", target_bir_lowering=False, debug=False)

    # Chunk-major layouts (prepared on host): tensor[p, kd, ...] holds row
    # kd*128+p of the logical matrix, so each loads as ONE big DMA with
    # multi-KB contiguous runs per partition (per-DMA fixed costs are
    # ~1.3 us; 128KB-tile loads would pay ~60% overhead).
    xTc = nc.dram_tensor("xTc", [128, NKD, S], F16, kind="ExternalInput").ap()
    wqc = nc.dram_tensor("wqc", [128, NKD, QC], F16, kind="ExternalInput").ap()
    wkc = nc.dram_tensor("wkc", [128, NKD, KC], F16, kind="ExternalInput").ap()
    wvc = nc.dram_tensor("wvc", [128, NKD, KC], F16, kind="ExternalInput").ap()
    woc = nc.dram_tensor("woc", [128, NPR, D], F16, kind="ExternalInput").ap()
    bq = nc.dram_tensor("bq", [QC], F32, kind="ExternalInput").ap()
    bk = nc.dram_tensor("bk", [KC], F32, kind="ExternalInput").ap()
    bvh = nc.dram_tensor("bvh", [KC], F16, kind="ExternalInput").ap()
    outT = nc.dram_tensor("outT", [D, S], F16, kind="ExternalOutput").ap()

    with tile.TileContext(nc) as tc, ExitStack() as ctx:
        singles = ctx.enter_context(tc.tile_pool(name="singles", bufs=1))
        wpool = ctx.enter_context(tc.tile_pool(name="wpool", bufs=1))
        xtp = ctx.enter_context(tc.tile_pool(name="xtp", bufs=2))
        qtp = ctx.enter_context(tc.tile_pool(name="qtp", bufs=2))
        ptp = ctx.enter_context(tc.tile_pool(name="ptp", bufs=2))
        afp = ctx.enter_context(tc.tile_pool(name="afp", bufs=2))
        atp = ctx.enter_context(tc.tile_pool(name="atp", bufs=4))
        otp = ctx.enter_context(tc.tile_pool(name="otp", bufs=3))
        rcpool = ctx.enter_context(tc.tile_pool(name="rcpool", bufs=8))

        pp_pj = ctx.enter_context(tc.tile_pool(name="pp_pj", bufs=2, space="PSUM"))
        pp_sc = ctx.enter_context(tc.tile_pool(name="pp_sc", bufs=2, space="PSUM"))
        pp_av = ctx.enter_context(tc.tile_pool(name="pp_av", bufs=2, space="PSUM"))

        # ---- constants ----
        ident = singles.tile([128, 128], F16, name="ident", tag="ident")
        make_identity(nc, ident)

        # maskNEG[t, c] = 0 where c >= t else MASK_NEG (applied to the
        # 128x128 diagonal sub-block of diagonal score chunks via a PE
        # matmul: ident.T @ maskNEG accumulated into the score group).
        maskNEG = singles.tile([128, 128], F16, name="maskNEG", tag="maskNEG")
        nc.gpsimd.memset(maskNEG, 0.0)
        nc.gpsimd.affine_select(
            out=maskNEG,
            in_=maskNEG,
            compare_op=mybir.AluOpType.is_ge,
            fill=MASK_NEG,
            base=0,
            pattern=[[1, 128]],
            channel_multiplier=-1,
        )

        ones_row = singles.tile([1, 128], F16, name="ones_row", tag="ones_row")
        nc.vector.memset(ones_row, 1.0)

        sbq = singles.tile([128, NQT], F32, name="sbq", tag="sbq")
        nc.sync.dma_start(out=sbq, in_=bq.rearrange("(t p) -> p t", p=128))
        sbk = singles.tile([128, 1], F32, name="sbk", tag="sbk")
        nc.sync.dma_start(out=sbk, in_=bk.rearrange("(t p) -> p t", p=128))
        bv_row = singles.tile([1, KC], F16, name="bv_row", tag="bv_row")
        nc.sync.dma_start(out=bv_row, in_=bvh.rearrange("(a k) -> a k", a=1))

        # ---- persistent weight tiles, one DMA per tensor ----
        wq_all = wpool.tile([128, NKD, QC], F16, name="wq_all", tag="wq_all")
        wk_all = wpool.tile([128, NKD, KC], F16, name="wk_all", tag="wk_all")
        wv_all = wpool.tile([128, NKD, KC], F16, name="wv_all", tag="wv_all")
        wo_all = wpool.tile([128, NPR, D], F16, name="wo_all", tag="wo_all")
        wq_t = [wq_all[:, kd, :] for kd in range(NKD)]
        wk_t = [wk_all[:, kd, :] for kd in range(NKD)]
        wv_t = [wv_all[:, kd, :] for kd in range(NKD)]
        wo_t = [wo_all[:, pr, :] for pr in range(NPR)]

        def emit_wo_dma():
            nc.sync.dma_start(out=wo_all, in_=woc)

        # ---- persistent K^T (per block) and V_aug ----
        kTb = []
        for j in range(NB):
            t = wpool.tile([128, SB], F16, name=f"kTb{j}", tag=f"kTb{j}")
            kTb.append(t)
        # vaug[g][j][:, tt, 0:64] = V rows for t-chunk (j*TPB+tt), group g;
        # col 64 = ones (folds the softmax denominator into the AV matmul).
        vaug = [[None] * NB for _ in range(GPC)]
        for g in range(GPC):
            for j in range(NB):
                t = wpool.tile(
                    [128, TPB, DK + 1], F16,
                    name=f"vaug{g}_{j}", tag=f"vaug{g}_{j}",
                )
                nc.vector.memset(t[:, :, DK:DK + 1], 1.0)
                vaug[g][j] = t

        xt_tiles = {}

        def emit_xt_dma(j):
            s0 = j * SB
            xt_all = xtp.tile([128, NKD, SB], F16, name="xt_all", tag="xt_all")
            nc.sync.dma_start(out=xt_all, in_=xTc[:, :, s0:s0 + SB])
            xt_tiles[j] = [xt_all[:, kd, :] for kd in range(NKD)]

        qT_tiles = {}

        def emit_proj_parts(j):
            """Returns a list of closures, each emitting one projection
            chain for block j (4 Q tiles, 1 K tile, 4 V t-tiles)."""
            xt = xt_tiles[j]
            qT = [None] * NQT
            qT_tiles[j] = qT
            parts = []

            def q_part(qt):
                def emit():
                    ps = pp_pj.tile([128, SB], F32, name="ps_q", tag="pj")
                    for kd in range(NKD):
                        nc.tensor.matmul(
                            out=ps,
                            lhsT=wq_t[kd][:, qt * 128:(qt + 1) * 128],
                            rhs=xt[kd],
                            start=(kd == 0),
                            stop=(kd == NKD - 1),
                        )
                    t = qtp.tile([128, SB], F16, name=f"qT{qt}", tag=f"qT{qt}")
                    nc.vector.tensor_scalar_add(
                        out=t, in0=ps, scalar1=sbq[:, qt:qt + 1]
                    )
                    qT[qt] = t
                return emit

            def k_part():
                def emit():
                    ps = pp_pj.tile([128, SB], F32, name="ps_k", tag="pj")
                    for kd in range(NKD):
                        nc.tensor.matmul(
                            out=ps, lhsT=wk_t[kd], rhs=xt[kd],
                            start=(kd == 0), stop=(kd == NKD - 1),
                        )
                    nc.vector.tensor_scalar_add(
                        out=kTb[j], in0=ps, scalar1=sbk
                    )
                return emit

            def v_part(tt):
                def emit():
                    # V in natural [t, kv] layout: x chunk stationary.
                    ps = pp_pj.tile([128, SB], F32, name="ps_v", tag="pj")
                    nc.tensor.matmul(
                        out=ps[:, 0:KC], lhsT=ones_row, rhs=bv_row,
                        start=True, stop=False,
                    )
                    for kd in range(NKD):
                        nc.tensor.matmul(
                            out=ps[:, 0:KC],
                            lhsT=xt[kd][:, tt * 128:(tt + 1) * 128],
                            rhs=wv_t[kd],
                            start=False,
                            stop=(kd == NKD - 1),
                        )
                    for g in range(GPC):
                        nc.vector.tensor_copy(
                            out=vaug[g][j][:, tt, 0:DK],
                            in_=ps[:, g * DK:(g + 1) * DK],
                        )
                return emit

            # K and Q0 first: the next block's first head's scores need
            # them; V feeds that head's AV phase just after.
            parts.append(k_part())
            parts.append(q_part(0))
            for tt in range(TPB):
                parts.append(v_part(tt))
            for qt in range(1, NQT):
                parts.append(q_part(qt))
            return parts

        apairs_by_block = {}

        def emit_oproj_parts(j):
            """Out-projection of block j: 16 column-tile closures."""
            s0 = j * SB
            apairs = apairs_by_block[j]

            osb4 = [None]

            def o_part(ot):
                def emit():
                    ps_o = pp_pj.tile([128, SB], F32, name="ps_o", tag="pj")
                    for pr in range(NPR):
                        nc.tensor.matmul(
                            out=ps_o,
                            lhsT=wo_t[pr][:, ot * 128:(ot + 1) * 128],
                            rhs=apairs[pr],
                            start=(pr == 0),
                            stop=(pr == NPR - 1),
                        )
                    if ot % 4 == 0:
                        osb4[0] = otp.tile([128, 4, SB], F16, name="osb",
                                           tag="osb")
                    nc.vector.tensor_copy(out=osb4[0][:, ot % 4, :], in_=ps_o)
                    if ot % 4 == 3:
                        # one DMA per 4 column tiles (d rows ot-3..ot)
                        dst = outT[(ot - 3) * 128:(ot + 1) * 128,
                                   s0:s0 + SB]
                        nc.sync.dma_start(
                            out=dst.rearrange("(o p) s -> p o s", p=128),
                            in_=osb4[0],
                        )
                return emit

            return [o_part(ot) for ot in range(NOT)]

        def emit_attention(j, fillers):
            """Attention for block j. `fillers` is a list of closures
            (PE-heavy, dependency-free work) drained into the stream to
            fill Act-bound stalls."""
            nti = TPB * (j + 1)
            qT = qT_tiles[j]
            # Spread fillers evenly over the block's fill slots (one slot
            # per (head, s-tile)) so later heads aren't starved.
            n_slots = HL * TPB
            fi = [0]
            slot = [0]

            def fill(last=False):
                slot[0] += 1
                want = len(fillers) if last else (
                    len(fillers) * slot[0] + n_slots - 1) // n_slots
                while fi[0] < min(want, len(fillers)):
                    fillers[fi[0]]()
                    fi[0] += 1

            aflip = [
                afp.tile([128, TPB, GPC, DK], F16, name=f"af{r}", tag=f"af{r}")
                for r in range(REP)
            ]
            apairs = [
                atp.tile([128, SB], F16, name=f"ap{r}", tag=f"ap{r}")
                for r in range(REP)
            ]
            apairs_by_block[j] = apairs

            # Head order (g, r): g-major so that after head (1, r) both
            # group slices of aflip[r] are complete and can be transposed.
            for g in range(GPC):
                for r in range(REP):
                    qtile = qT[r]
                    qrow = g * DK
                    # Score chunks live in [128, 2, SB] pair tiles (2 PSUM
                    # banks); off-diagonal pairs share one 1024-col exp.
                    pair_pt = [None] * (nti // 2)
                    for ti in range(nti):
                        sub = ti % 2
                        if sub == 0:
                            ps_p = pp_sc.tile([128, 2, SB], F32, name="ps_p",
                                              tag="sc")
                            pt2 = ptp.tile([128, 2, SB], F16,
                                           name=f"pt{ti // 2}",
                                           tag=f"pt{ti // 2}")
                            pair_pt[ti // 2] = pt2
                        krel = ti - TPB * j
                        c0 = 128 * krel if krel > 0 else 0
                        nc.tensor.matmul(
                            out=ps_p[:, sub, c0:SB],
                            lhsT=kTb[ti // TPB][g * DK:(g + 1) * DK,
                                               (ti % TPB) * 128:
                                               (ti % TPB + 1) * 128],
                            rhs=qtile[qrow:qrow + DK, c0:SB],
                            start=True,
                            stop=(krel < 0),
                        )
                        if krel >= 0:
                            # diagonal 128x128: additive causal mask via PE
                            nc.tensor.matmul(
                                out=ps_p[:, sub, c0:c0 + 128],
                                lhsT=ident,
                                rhs=maskNEG,
                                start=False,
                                stop=True,
                            )
                            nc.scalar.activation(
                                out=pt2[:, sub, c0:SB],
                                in_=ps_p[:, sub, c0:SB],
                                func=mybir.ActivationFunctionType.Exp,
                                scale=0.125,
                            )
                        elif sub == 1:
                            # both chunks of an off-diagonal pair: one exp
                            nc.scalar.activation(
                                out=pt2[:, :, :], in_=ps_p[:, :, :],
                                func=mybir.ActivationFunctionType.Exp,
                                scale=0.125,
                            )

                    # AV per s-tile: P stationary, V_aug moving (N=65).
                    for st in range(TPB):
                        jst = TPB * j + st
                        ps_av = pp_av.tile([128, DK + 1], F32, name="ps_av",
                                           tag="av")
                        for ti in range(jst + 1):
                            nc.tensor.matmul(
                                out=ps_av,
                                lhsT=pair_pt[ti // 2][:, ti % 2,
                                                      st * 128:(st + 1) * 128],
                                rhs=vaug[g][ti // TPB][:, ti % TPB, :],
                                start=(ti == 0),
                                stop=(ti == jst),
                            )
                        rcp = rcpool.tile([128, 1], F32, name="rcp", tag="rcp")
                        nc.vector.reciprocal(out=rcp, in_=ps_av[:, DK:DK + 1])
                        nc.vector.tensor_scalar_mul(
                            out=aflip[r][:, st, g, :],
                            in0=ps_av[:, 0:DK],
                            scalar1=rcp,
                        )
                        fill()
                    if g == 1:
                        # aflip[r] complete: [s, (g, dk)] -> [hd, s] via
                        # SBUF->SBUF DMA transpose into the apair tile.
                        for st in range(TPB):
                            nc.sync.dma_start_transpose(
                                out=apairs[r][:, st * 128:(st + 1) * 128],
                                in_=aflip[r][:, st, :, :],
                            )
            fill(last=True)

        # ---- main schedule ----
        nc.sync.dma_start(out=wk_all, in_=wkc)
        emit_xt_dma(0)
        nc.sync.dma_start(out=wq_all, in_=wqc)
        nc.sync.dma_start(out=wv_all, in_=wvc)
        a0 = emit_proj_parts(0)
        for p in a0[:6]:   # K, Q0, V0-3 inline; Q1-3 become B_0 fillers
            p()
        emit_xt_dma(1)
        emit_attention(0, a0[6:] + emit_proj_parts(1)
                       + [lambda: emit_xt_dma(2), emit_wo_dma])
        c0 = emit_oproj_parts(0)
        emit_attention(1, emit_proj_parts(2) + [lambda: emit_xt_dma(3)]
                       + c0[:4])
        c1 = emit_oproj_parts(1)
        emit_attention(2, emit_proj_parts(3) + c1[:4])
        c2 = emit_oproj_parts(2)
        emit_attention(3, c0[4:] + c1[4:] + c2)
        for p in emit_oproj_parts(3):
            p()

    nc.compile()
    return nc


def make_in_maps(x, Wq, bq, Wk, bk, Wv, bv, Wo, bo):
    x = np.asarray(x, dtype=np.float32)
    Wq = np.asarray(Wq, dtype=np.float32)
    Wk = np.asarray(Wk, dtype=np.float32)
    Wv = np.asarray(Wv, dtype=np.float32)
    Wo = np.asarray(Wo, dtype=np.float32)
    bq = np.asarray(bq, dtype=np.float32)
    bk = np.asarray(bk, dtype=np.float32)
    bv = np.asarray(bv, dtype=np.float32)
    # Local-head layout permutation: q-tile m = [head m (g0) | head 4+m (g1)]
    perm = [0, REP, 1, REP + 1, 2, REP + 2, 3, REP + 3][:HL]
    in_maps = []
    for c in range(NCORES):
        b = c // (NCORES // B)
        gp = c % (NCORES // B)
        q0 = gp * QC
        k0 = gp * KC
        qcols = np.concatenate(
            [np.arange(q0 + hl * DK, q0 + (hl + 1) * DK) for hl in perm]
        )
        def chunk_major(m):
            # [R, C] -> [128, R//128, C]: out[p, kd, c] = m[kd*128+p, c]
            m = m.astype(np.float16)
            return np.ascontiguousarray(
                m.reshape(-1, 128, m.shape[1]).transpose(1, 0, 2)
            )

        in_maps.append({
            "xTc": chunk_major(x[b].T),
            "wqc": chunk_major(Wq[:, qcols]),
            "wkc": chunk_major(Wk[:, k0:k0 + KC]),
            "wvc": chunk_major(Wv[:, k0:k0 + KC]),
            "woc": chunk_major(Wo[qcols, :]),
            "bq": np.ascontiguousarray(bq[qcols]),
            "bk": np.ascontiguousarray(bk[k0:k0 + KC]),
            "bvh": np.ascontiguousarray(bv[k0:k0 + KC].astype(np.float16)),
        })
    return in_maps


def assemble_output(results, bo):
    bo = np.asarray(bo, dtype=np.float32)
    out = np.zeros((B, S, D), dtype=np.float32)
    for c in range(NCORES):
        b = c // (NCORES // B)
        out[b] += results[c]["outT"].T.astype(np.float32)
    out += bo
    return out


_NC_CACHE = None


def kernel(x, Wq, bq, Wk, bk, Wv, bv, Wo, bo):
    global _NC_CACHE
    from concourse.bass_utils import run_bass_kernel_spmd

    if _NC_CACHE is None:
        _NC_CACHE = build_gqa_nc()
    nc = _NC_CACHE
    in_maps = make_in_maps(x, Wq, bq, Wk, bk, Wv, bv, Wo, bo)
    res = run_bass_kernel_spmd(nc, in_maps, list(range(NCORES))).results
    return assemble_output(res, bo)


# revision 3
# speedup vs baseline: 1.1381x; 1.1006x over previous
"""Grouped-Query Attention kernel for Trainium2, 8-core SPMD. v2.

Problem (full shapes): B=2, S=2048, D=2048, H=32 q-heads, KV=8 kv-heads,
DK=64, REP=4.

Sharding: 16 (batch, kv-group) units over 8 cores -> each core owns one
batch b and 2 adjacent kv-groups (8 query heads, 512 q-cols / 128 kv-cols).
Each core computes its heads' attention output and a partial output
projection against its 512-row slice of Wo; the host sums the 4 partials
per batch and adds bo.

v2 restructuring vs v1 (418.7us -> 247.1us on the cost model):
- AV matmul flipped: out[s, dk+1] = P^T.T @ V_aug with the P tile as
  stationary (N=65 per chunk instead of 512) - halves AV PE cycles and
  puts the softmax denominator on the partition axis, so normalization
  is one reciprocal + one tensor_scalar_mul on DVE.
- Causal mask applied post-exp on the (otherwise idle) Pool engine:
  affine_select zeroes the upper triangle of the diagonal 128x128
  block of the f16 P tile in SBUF - no PE/DVE/Act mask work.
- Off-diagonal score chunks live in [128, 2, 512] 2-bank PSUM pair
  tiles; each pair shares one 1024-col exp (amortizes Act init).
- Attention output transposed back to [hd, s] with SBUF->SBUF DMA
  transposes (14 ns per 16x128 xbar tile) instead of PE transposes.
- V projected directly in [t, kv] layout (x chunk as stationary), bias
  folded in via a K=1 ones matmul - no V transposes.
- QKV bias adds on DVE (tensor_scalar_add), Act engine runs exps only.
- Software-pipelined emission: out-proj of block j-1 and QKV proj of
  block j+1 are interleaved (evenly spread) into the Act-bound
  attention phase of block j to keep the in-order PE stream fed;
  extra out-proj tiles are donated to the most Act-bound block 3.
- Inputs in chunk-major host layouts so each tensor loads as one big
  DMA (per-DMA fixed cost ~1.3us); block-0 x split in two halves with
  the K and Q0 chains interleaved across both pj PSUM banks so the
  cold start chases the DMA tail; outT written 4 column-tiles per DMA.
- f16 output partials (halves output DMA; host sums in f32).
"""

import os
from contextlib import ExitStack

import numpy as np

import concourse.bass as bass
import concourse.tile as tile
from concourse import bacc
from concourse import mybir

F32 = mybir.dt.float32
F16 = mybir.dt.float16

# Full-problem constants (hardcoded per contest contract).
B = 2
S = 2048
D = 2048
H = 32
KV = 8
DK = 64
REP = H // KV          # 4
NCORES = 8

GPC = (KV * B) // NCORES      # kv-groups per core = 2
QC = GPC * REP * DK           # local q cols = 512
KC = GPC * DK                 # local k cols = 128
HL = GPC * REP                # local heads = 8
SB = 512                      # s-block size
NB = S // SB                  # 4 blocks
NKD = D // 128                # 16 contraction chunks for projections
NQT = QC // 128               # 4 q-col tiles
NPR = QC // 128               # 4 head-pair tiles (rhs chunks for out proj)
NOT = D // 128                # 16 out-col tiles
TPB = SB // 128               # 4 t-chunks per s-block



def build_gqa_nc():
    nc = bacc.Bacc("TRN2", target_bir_lowering=False, debug=False)

    # Chunk-major layouts (prepared on host): tensor[p, kd, ...] holds row
    # kd*128+p of the logical matrix, so each loads as ONE big DMA with
    # multi-KB contiguous runs per partition (per-DMA fixed costs are
    # ~1.3 us; 128KB-tile loads would pay ~60% overhead).
    xTc = nc.dram_tensor("xTc", [128, NKD, S], F16, kind="ExternalInput").ap()
    wqc = nc.dram_tensor(
        "wqc", [NQT, 128, NKD, 128], F16, kind="ExternalInput"
    ).ap()
    wkc = nc.dram_tensor("wkc", [128, NKD, KC], F16, kind="ExternalInput").ap()
    wvc = nc.dram_tensor("wvc", [128, NKD, KC], F16, kind="ExternalInput").ap()
    woc = nc.dram_tensor("woc", [128, NPR, D], F16, kind="ExternalInput").ap()
    bq = nc.dram_tensor("bq", [QC], F32, kind="ExternalInput").ap()
    bk = nc.dram_tensor("bk", [KC], F32, kind="ExternalInput").ap()
    bvh = nc.dram_tensor("bvh", [KC], F16, kind="ExternalInput").ap()
    outT = nc.dram_tensor("outT", [D, S], F16, kind="ExternalOutput").ap()

    with tile.TileContext(nc) as tc, ExitStack() as ctx:
        singles = ctx.enter_context(tc.tile_pool(name="singles", bufs=1))
        wpool = ctx.enter_context(tc.tile_pool(name="wpool", bufs=1))
        xtp = ctx.enter_context(tc.tile_pool(name="xtp", bufs=2))
        qtp = ctx.enter_context(tc.tile_pool(name="qtp", bufs=2))
        ptp = ctx.enter_context(tc.tile_pool(name="ptp", bufs=2))
        afp = ctx.enter_context(tc.tile_pool(name="afp", bufs=2))
        atp = ctx.enter_context(tc.tile_pool(name="atp", bufs=4))
        otp = ctx.enter_context(tc.tile_pool(name="otp", bufs=3))
        rcpool = ctx.enter_context(tc.tile_pool(name="rcpool", bufs=8))

        pp_pj = ctx.enter_context(tc.tile_pool(name="pp_pj", bufs=2, space="PSUM"))
        pp_sc = ctx.enter_context(tc.tile_pool(name="pp_sc", bufs=2, space="PSUM"))
        pp_av = ctx.enter_context(tc.tile_pool(name="pp_av", bufs=2, space="PSUM"))

        # ---- constants ----
        ones_row = singles.tile([1, 128], F16, name="ones_row", tag="ones_row")
        nc.vector.memset(ones_row, 1.0)

        sbq = singles.tile([128, NQT], F32, name="sbq", tag="sbq")
        nc.sync.dma_start(out=sbq, in_=bq.rearrange("(t p) -> p t", p=128))
        sbk = singles.tile([128, 1], F32, name="sbk", tag="sbk")
        nc.sync.dma_start(out=sbk, in_=bk.rearrange("(t p) -> p t", p=128))
        bv_row = singles.tile([1, KC], F16, name="bv_row", tag="bv_row")
        nc.sync.dma_start(out=bv_row, in_=bvh.rearrange("(a k) -> a k", a=1))

        # ---- persistent weight tiles, one DMA per tensor (wq: one per
        # q-tile, so the first Q chain starts after 0.5MB not 2MB) ----
        wq_qt = [
            wpool.tile([128, NKD, 128], F16, name=f"wq_qt{qt}", tag=f"wq{qt}")
            for qt in range(NQT)
        ]
        wk_all = wpool.tile([128, NKD, KC], F16, name="wk_all", tag="wk_all")
        wv_all = wpool.tile([128, NKD, KC], F16, name="wv_all", tag="wv_all")
        wo_all = wpool.tile([128, NPR, D], F16, name="wo_all", tag="wo_all")
        wk_t = [wk_all[:, kd, :] for kd in range(NKD)]
        wv_t = [wv_all[:, kd, :] for kd in range(NKD)]
        wo_t = [wo_all[:, pr, :] for pr in range(NPR)]

        def emit_wq_dma(qt):
            nc.sync.dma_start(out=wq_qt[qt], in_=wqc[qt])

        def emit_wo_dma():
            nc.sync.dma_start(out=wo_all, in_=woc)

        # ---- persistent K^T (per block) and V_aug ----
        kTb = []
        for j in range(NB):
            t = wpool.tile([128, SB], F16, name=f"kTb{j}", tag=f"kTb{j}")
            kTb.append(t)
        # vaug[g][j][:, tt, 0:64] = V rows for t-chunk (j*TPB+tt), group g;
        # col 64 = ones (folds the softmax denominator into the AV matmul).
        vaug = [[None] * NB for _ in range(GPC)]
        for g in range(GPC):
            for j in range(NB):
                t = wpool.tile(
                    [128, TPB, DK + 1], F16,
                    name=f"vaug{g}_{j}", tag=f"vaug{g}_{j}",
                )
                nc.vector.memset(t[:, :, DK:DK + 1], 1.0)
                vaug[g][j] = t

        xt_tiles = {}

        def emit_xt_dma(j, split=False):
            s0 = j * SB
            xt_all = xtp.tile([128, NKD, SB], F16, name="xt_all", tag="xt_all")
            if split:
                h = NKD // 2
                nc.sync.dma_start(
                    out=xt_all[:, 0:h, :], in_=xTc[:, 0:h, s0:s0 + SB]
                )
                nc.sync.dma_start(
                    out=xt_all[:, h:NKD, :], in_=xTc[:, h:NKD, s0:s0 + SB]
                )
            else:
                nc.sync.dma_start(out=xt_all, in_=xTc[:, :, s0:s0 + SB])
            xt_tiles[j] = [xt_all[:, kd, :] for kd in range(NKD)]

        qT_tiles = {}

        def emit_proj_parts(j):
            """Returns a list of closures, each emitting one projection
            chain for block j (4 Q tiles, 1 K tile, 4 V t-tiles)."""
            xt = xt_tiles[j]
            qT = [None] * NQT
            qT_tiles[j] = qT
            parts = []

            def q_part(qt):
                def emit():
                    ps = pp_pj.tile([128, SB], F32, name="ps_q", tag="pj")
                    for kd in range(NKD):
                        nc.tensor.matmul(
                            out=ps,
                            lhsT=wq_qt[qt][:, kd, :],
                            rhs=xt[kd],
                            start=(kd == 0),
                            stop=(kd == NKD - 1),
                        )
                    t = qtp.tile([128, SB], F16, name=f"qT{qt}", tag=f"qT{qt}")
                    nc.vector.tensor_scalar_add(
                        out=t, in0=ps, scalar1=sbq[:, qt:qt + 1]
                    )
                    qT[qt] = t
                return emit

            def k_part():
                def emit():
                    ps = pp_pj.tile([128, SB], F32, name="ps_k", tag="pj")
                    for kd in range(NKD):
                        nc.tensor.matmul(
                            out=ps, lhsT=wk_t[kd], rhs=xt[kd],
                            start=(kd == 0), stop=(kd == NKD - 1),
                        )
                    nc.vector.tensor_scalar_add(
                        out=kTb[j], in0=ps, scalar1=sbk
                    )
                return emit

            def v_part(tt):
                def emit():
                    # V in natural [t, kv] layout: x chunk stationary.
                    ps = pp_pj.tile([128, SB], F32, name="ps_v", tag="pj")
                    nc.tensor.matmul(
                        out=ps[:, 0:KC], lhsT=ones_row, rhs=bv_row,
                        start=True, stop=False,
                    )
                    for kd in range(NKD):
                        nc.tensor.matmul(
                            out=ps[:, 0:KC],
                            lhsT=xt[kd][:, tt * 128:(tt + 1) * 128],
                            rhs=wv_t[kd],
                            start=False,
                            stop=(kd == NKD - 1),
                        )
                    for g in range(GPC):
                        nc.vector.tensor_copy(
                            out=vaug[g][j][:, tt, 0:DK],
                            in_=ps[:, g * DK:(g + 1) * DK],
                        )
                return emit

            def kq0_part():
                # K and Q0 chains interleaved by kd (both pj banks): both
                # finish chasing the x DMA tail instead of serializing.
                def emit():
                    ps_k = pp_pj.tile([128, SB], F32, name="ps_k", tag="pj")
                    ps_q = pp_pj.tile([128, SB], F32, name="ps_q", tag="pj")
                    for kd in range(NKD):
                        nc.tensor.matmul(
                            out=ps_k, lhsT=wk_t[kd], rhs=xt[kd],
                            start=(kd == 0), stop=(kd == NKD - 1),
                        )
                        nc.tensor.matmul(
                            out=ps_q, lhsT=wq_qt[0][:, kd, :], rhs=xt[kd],
                            start=(kd == 0), stop=(kd == NKD - 1),
                        )
                    nc.vector.tensor_scalar_add(
                        out=kTb[j], in0=ps_k, scalar1=sbk
                    )
                    t = qtp.tile([128, SB], F16, name="qT0", tag="qT0")
                    nc.vector.tensor_scalar_add(
                        out=t, in0=ps_q, scalar1=sbq[:, 0:1]
                    )
                    qT[0] = t
                return emit

            if j == 0:
                # cold start: fused K||Q0; V parts go to pre-AV fillers
                parts.append(kq0_part())
            else:
                parts.append(k_part())
                parts.append(q_part(0))
            for tt in range(TPB):
                parts.append(v_part(tt))
            for qt in range(1, NQT):
                parts.append(q_part(qt))
            return parts

        apairs_by_block = {}

        def emit_oproj_parts(j, batch=4):
            """Out-projection of block j: 16 column-tile closures. outT
            writes are DMA'd `batch` column-tiles at a time."""
            s0 = j * SB
            apairs = apairs_by_block[j]

            osb4 = [None]

            def o_part(ot):
                def emit():
                    ps_o = pp_pj.tile([128, SB], F32, name="ps_o", tag="pj")
                    for pr in range(NPR):
                        nc.tensor.matmul(
                            out=ps_o,
                            lhsT=wo_t[pr][:, ot * 128:(ot + 1) * 128],
                            rhs=apairs[pr],
                            start=(pr == 0),
                            stop=(pr == NPR - 1),
                        )
                    sub = ot % batch
                    if sub == 0:
                        osb4[0] = otp.tile([128, 4, SB], F16, name="osb",
                                           tag="osb")
                    nc.vector.tensor_copy(out=osb4[0][:, sub, :], in_=ps_o)
                    if sub == batch - 1:
                        dst = outT[(ot - sub) * 128:(ot + 1) * 128,
                                   s0:s0 + SB]
                        nc.sync.dma_start(
                            out=dst.rearrange("(o p) s -> p o s", p=128),
                            in_=osb4[0][:, 0:batch, :],
                        )
                return emit

            return [o_part(ot) for ot in range(NOT)]

        def emit_attention(j, fillers, pre_av=()):
            """Attention for block j. `fillers` is a list of closures
            (PE-heavy, dependency-free work) drained into the stream to
            fill Act-bound stalls. `pre_av` closures are emitted after the
            first head's score chunks, before its AV groups (used for
            block 0's V parts, which that AV phase depends on)."""
            nti = TPB * (j + 1)
            qT = qT_tiles[j]
            # Spread fillers evenly over the block's fill slots (one slot
            # per (head, s-tile)) so later heads aren't starved.
            n_slots = HL * TPB
            fi = [0]
            slot = [0]

            def fill(last=False):
                slot[0] += 1
                want = len(fillers) if last else (
                    len(fillers) * slot[0] + n_slots - 1) // n_slots
                while fi[0] < min(want, len(fillers)):
                    fillers[fi[0]]()
                    fi[0] += 1

            aflip = [
                afp.tile([128, TPB, GPC, DK], F16, name=f"af{r}", tag=f"af{r}")
                for r in range(REP)
            ]
            apairs = [
                atp.tile([128, SB], F16, name=f"ap{r}", tag=f"ap{r}")
                for r in range(REP)
            ]
            apairs_by_block[j] = apairs

            # Head order (g, r): g-major so that after head (1, r) both
            # group slices of aflip[r] are complete and can be transposed.
            for g in range(GPC):
                for r in range(REP):
                    qtile = qT[r]
                    qrow = g * DK
                    # Score chunks live in [128, 2, SB] pair tiles (2 PSUM
                    # banks); off-diagonal pairs share one 1024-col exp.
                    pair_pt = [None] * (nti // 2)
                    for ti in range(nti):
                        sub = ti % 2
                        if sub == 0:
                            ps_p = pp_sc.tile([128, 2, SB], F32, name="ps_p",
                                              tag="sc")
                            pt2 = ptp.tile([128, 2, SB], F16,
                                           name=f"pt{ti // 2}",
                                           tag=f"pt{ti // 2}")
                            pair_pt[ti // 2] = pt2
                        krel = ti - TPB * j
                        c0 = 128 * krel if krel > 0 else 0
                        nc.tensor.matmul(
                            out=ps_p[:, sub, c0:SB],
                            lhsT=kTb[ti // TPB][g * DK:(g + 1) * DK,
                                               (ti % TPB) * 128:
                                               (ti % TPB + 1) * 128],
                            rhs=qtile[qrow:qrow + DK, c0:SB],
                            start=True,
                            stop=True,
                        )
                        if krel >= 0:
                            nc.scalar.activation(
                                out=pt2[:, sub, c0:SB],
                                in_=ps_p[:, sub, c0:SB],
                                func=mybir.ActivationFunctionType.Exp,
                                scale=0.125,
                            )
                            # causal mask: zero the upper triangle of the
                            # diagonal 128x128 block post-exp, on Pool
                            # (keep where col - row >= 0, else fill 0)
                            nc.gpsimd.affine_select(
                                out=pt2[:, sub, c0:c0 + 128],
                                in_=pt2[:, sub, c0:c0 + 128],
                                compare_op=mybir.AluOpType.is_ge,
                                fill=0.0,
                                base=0,
                                pattern=[[1, 128]],
                                channel_multiplier=-1,
                            )
                        elif sub == 1:
                            # both chunks of an off-diagonal pair: one exp
                            nc.scalar.activation(
                                out=pt2[:, :, :], in_=ps_p[:, :, :],
                                func=mybir.ActivationFunctionType.Exp,
                                scale=0.125,
                            )

                    if g == 0 and r == 0:
                        for p in pre_av:
                            p()

                    # AV per s-tile: P stationary, V_aug moving (N=65).
                    for st in range(TPB):
                        jst = TPB * j + st
                        ps_av = pp_av.tile([128, DK + 1], F32, name="ps_av",
                                           tag="av")
                        for ti in range(jst + 1):
                            nc.tensor.matmul(
                                out=ps_av,
                                lhsT=pair_pt[ti // 2][:, ti % 2,
                                                      st * 128:(st + 1) * 128],
                                rhs=vaug[g][ti // TPB][:, ti % TPB, :],
                                start=(ti == 0),
                                stop=(ti == jst),
                            )
                        rcp = rcpool.tile([128, 1], F32, name="rcp", tag="rcp")
                        nc.vector.reciprocal(out=rcp, in_=ps_av[:, DK:DK + 1])
                        nc.vector.tensor_scalar_mul(
                            out=aflip[r][:, st, g, :],
                            in0=ps_av[:, 0:DK],
                            scalar1=rcp,
                        )
                        fill()
                    if g == 1:
                        # aflip[r] complete: [s, (g, dk)] -> [hd, s] via
                        # SBUF->SBUF DMA transpose into the apair tile.
                        for st in range(TPB):
                            nc.sync.dma_start_transpose(
                                out=apairs[r][:, st * 128:(st + 1) * 128],
                                in_=aflip[r][:, st, :, :],
                            )
            fill(last=True)

        # ---- main schedule ----
        nc.sync.dma_start(out=wk_all, in_=wkc)
        emit_xt_dma(0, split=True)
        emit_wq_dma(0)
        nc.sync.dma_start(out=wv_all, in_=wvc)
        for qt in range(1, NQT):
            emit_wq_dma(qt)
        a0 = emit_proj_parts(0)
        a0[0]()            # fused K||Q0; V0-3 run as h0 pre-AV fillers
        emit_xt_dma(1)
        emit_attention(0, a0[5:] + emit_proj_parts(1)
                       + [lambda: emit_xt_dma(2), emit_wo_dma],
                       pre_av=a0[1:5])
        c0 = emit_oproj_parts(0)
        emit_attention(1, emit_proj_parts(2) + [lambda: emit_xt_dma(3)]
                       + c0[:4])
        c1 = emit_oproj_parts(1)
        emit_attention(2, emit_proj_parts(3) + c1[:4])
        c2 = emit_oproj_parts(2)
        emit_attention(3, c0[4:] + c1[4:] + c2)
        for p in emit_oproj_parts(3, batch=2):
            p()

    nc.compile()
    return nc


def make_in_maps(x, Wq, bq, Wk, bk, Wv, bv, Wo, bo):
    x = np.asarray(x, dtype=np.float32)
    Wq = np.asarray(Wq, dtype=np.float32)
    Wk = np.asarray(Wk, dtype=np.float32)
    Wv = np.asarray(Wv, dtype=np.float32)
    Wo = np.asarray(Wo, dtype=np.float32)
    bq = np.asarray(bq, dtype=np.float32)
    bk = np.asarray(bk, dtype=np.float32)
    bv = np.asarray(bv, dtype=np.float32)
    # Local-head layout permutation: q-tile m = [head m (g0) | head 4+m (g1)]
    perm = [0, REP, 1, REP + 1, 2, REP + 2, 3, REP + 3][:HL]
    in_maps = []
    for c in range(NCORES):
        b = c // (NCORES // B)
        gp = c % (NCORES // B)
        q0 = gp * QC
        k0 = gp * KC
        qcols = np.concatenate(
            [np.arange(q0 + hl * DK, q0 + (hl + 1) * DK) for hl in perm]
        )
        def chunk_major(m):
            # [R, C] -> [128, R//128, C]: out[p, kd, c] = m[kd*128+p, c]
            m = m.astype(np.float16)
            return np.ascontiguousarray(
                m.reshape(-1, 128, m.shape[1]).transpose(1, 0, 2)
            )

        wq_cm = chunk_major(Wq[:, qcols])  # [128, NKD, QC]
        in_maps.append({
            "xTc": chunk_major(x[b].T),
            "wqc": np.ascontiguousarray(
                wq_cm.reshape(128, NKD, NQT, 128).transpose(2, 0, 1, 3)
            ),
            "wkc": chunk_major(Wk[:, k0:k0 + KC]),
            "wvc": chunk_major(Wv[:, k0:k0 + KC]),
            "woc": chunk_major(Wo[qcols, :]),
            "bq": np.ascontiguousarray(bq[qcols]),
            "bk": np.ascontiguousarray(bk[k0:k0 + KC]),
            "bvh": np.ascontiguousarray(bv[k0:k0 + KC].astype(np.float16)),
        })
    return in_maps


def assemble_output(results, bo):
    bo = np.asarray(bo, dtype=np.float32)
    out = np.zeros((B, S, D), dtype=np.float32)
    for c in range(NCORES):
        b = c // (NCORES // B)
        out[b] += results[c]["outT"].T.astype(np.float32)
    out += bo
    return out


_NC_CACHE = None


def kernel(x, Wq, bq, Wk, bk, Wv, bv, Wo, bo):
    global _NC_CACHE
    from concourse.bass_utils import run_bass_kernel_spmd

    if _NC_CACHE is None:
        _NC_CACHE = build_gqa_nc()
    nc = _NC_CACHE
    in_maps = make_in_maps(x, Wq, bq, Wk, bk, Wv, bv, Wo, bo)
    res = run_bass_kernel_spmd(nc, in_maps, list(range(NCORES))).results
    return assemble_output(res, bo)


# revision 4
# speedup vs baseline: 1.1437x; 1.0048x over previous
"""Grouped-Query Attention kernel for Trainium2, 8-core SPMD. v2.

Problem (full shapes): B=2, S=2048, D=2048, H=32 q-heads, KV=8 kv-heads,
DK=64, REP=4.

Sharding: 16 (batch, kv-group) units over 8 cores -> each core owns one
batch b and 2 adjacent kv-groups (8 query heads, 512 q-cols / 128 kv-cols).
Each core computes its heads' attention output and a partial output
projection against its 512-row slice of Wo; the host sums the 4 partials
per batch and adds bo.

v2/v3 restructuring vs v1 (418.7us -> 224.6us on the cost model):
- Q/K/V and output projections run as fp8e4m3 DoubleRow matmuls (2
  K-chunks per instruction at 0.5 cyc/row = 4x f16 throughput) with
  hi/lo error compensation: W*32 split into fp8 hi + residual lo on
  the host, three passes (hi*hi + lo*hi + hi*lo) accumulate in PSUM,
  and the DVE epilogue rescales by 1/32. Attention output hi/lo fp8
  bytes are packed into uint16 lanes so one 2-byte DMA transpose
  carries both halves; the out-proj reads them as stride-2 fp8 APs.
  Scores/AV stay f16 (softmax is error-sensitive; K=64 contraction
  cannot use DoubleRow anyway). rel_err 7e-4 -> 6.5e-3, gate 2e-2.
- AV matmul flipped: out[s, dk+1] = P^T.T @ V_aug with the P tile as
  stationary (N=65 per chunk instead of 512) - halves AV PE cycles and
  puts the softmax denominator on the partition axis, so normalization
  is one reciprocal + one tensor_scalar_mul on DVE.
- Causal mask applied post-exp on the (otherwise idle) Pool engine:
  affine_select zeroes the upper triangle of the diagonal 128x128
  block of the f16 P tile in SBUF - no PE/DVE/Act mask work.
- Off-diagonal score chunks live in [128, 2, 512] 2-bank PSUM pair
  tiles; each pair shares one 1024-col exp (amortizes Act init).
- Attention output transposed back to [hd, s] with SBUF->SBUF DMA
  transposes (14 ns per 16x128 xbar tile) instead of PE transposes.
- V projected directly in [t, kv] layout (x chunk as stationary), bias
  folded in via a K=1 ones matmul - no V transposes.
- QKV bias adds on DVE (tensor_scalar_add), Act engine runs exps only.
- Software-pipelined emission: out-proj of block j-1 and QKV proj of
  block j+1 are interleaved (evenly spread) into the Act-bound
  attention phase of block j to keep the in-order PE stream fed;
  extra out-proj tiles are donated to the most Act-bound block 3.
- Inputs in chunk-major host layouts so each tensor loads as one big
  DMA (per-DMA fixed cost ~1.3us); block-0 x split in two halves with
  the K and Q0 chains interleaved across both pj PSUM banks so the
  cold start chases the DMA tail; outT written 4 column-tiles per DMA.
- f16 output partials (halves output DMA; host sums in f32).
"""

import os
from contextlib import ExitStack

import numpy as np

import concourse.bass as bass
import concourse.tile as tile
from concourse import bacc
from concourse import mybir

F32 = mybir.dt.float32
F16 = mybir.dt.float16
F8 = mybir.dt.float8e4

# fp8 hi/lo compensated projections: W pre-scaled by SCL on the host so
# W*SCL sits in e4m3's normal range; the DVE epilogue multiplies by 1/SCL.
SCL = 32.0
ISCL = 1.0 / SCL

# Full-problem constants (hardcoded per contest contract).
B = 2
S = 2048
D = 2048
H = 32
KV = 8
DK = 64
REP = H // KV          # 4
NCORES = 8

GPC = (KV * B) // NCORES      # kv-groups per core = 2
QC = GPC * REP * DK           # local q cols = 512
KC = GPC * DK                 # local k cols = 128
HL = GPC * REP                # local heads = 8
SB = 512                      # s-block size
NB = S // SB                  # 4 blocks
NKD = D // 128                # 16 contraction chunks for projections
NQT = QC // 128               # 4 q-col tiles
NPR = QC // 128               # 4 head-pair tiles (rhs chunks for out proj)
NOT = D // 128                # 16 out-col tiles
TPB = SB // 128               # 4 t-chunks per s-block



def build_gqa_nc():
    nc = bacc.Bacc("TRN2", target_bir_lowering=False, debug=False)

    # Chunk-major layouts (prepared on host): tensor[p, kd, ...] holds row
    # kd*128+p of the logical matrix, so each loads as ONE big DMA with
    # multi-KB contiguous runs per partition (per-DMA fixed costs are
    # ~1.3 us; 128KB-tile loads would pay ~60% overhead).
    # fp8 hi/lo pairs (dim of size 2 = hi/lo) for the QKV projections;
    # total bytes match the old f16 layouts.
    xTc = nc.dram_tensor(
        "xTc", [128, 2, NKD, S], F8, kind="ExternalInput"
    ).ap()
    wqc = nc.dram_tensor(
        "wqc", [NQT, 2, 128, NKD, 128], F8, kind="ExternalInput"
    ).ap()
    wkc = nc.dram_tensor(
        "wkc", [2, 128, NKD, KC], F8, kind="ExternalInput"
    ).ap()
    wvc = nc.dram_tensor(
        "wvc", [2, 128, NKD, KC], F8, kind="ExternalInput"
    ).ap()
    woc = nc.dram_tensor(
        "woc", [2, 128, NPR, D], F8, kind="ExternalInput"
    ).ap()
    bq = nc.dram_tensor("bq", [QC], F32, kind="ExternalInput").ap()
    bk = nc.dram_tensor("bk", [KC], F32, kind="ExternalInput").ap()
    bvh = nc.dram_tensor("bvh", [KC], F16, kind="ExternalInput").ap()
    outT = nc.dram_tensor("outT", [D, S], F16, kind="ExternalOutput").ap()

    with tile.TileContext(nc) as tc, ExitStack() as ctx:
        singles = ctx.enter_context(tc.tile_pool(name="singles", bufs=1))
        wpool = ctx.enter_context(tc.tile_pool(name="wpool", bufs=1))
        xtp = ctx.enter_context(tc.tile_pool(name="xtp", bufs=2))
        qtp = ctx.enter_context(tc.tile_pool(name="qtp", bufs=2))
        ptp = ctx.enter_context(tc.tile_pool(name="ptp", bufs=2))
        afp = ctx.enter_context(tc.tile_pool(name="afp", bufs=2))
        atp = ctx.enter_context(tc.tile_pool(name="atp", bufs=4))
        otp = ctx.enter_context(tc.tile_pool(name="otp", bufs=3))
        rcpool = ctx.enter_context(tc.tile_pool(name="rcpool", bufs=8))

        pp_pj = ctx.enter_context(tc.tile_pool(name="pp_pj", bufs=2, space="PSUM"))
        pp_sc = ctx.enter_context(tc.tile_pool(name="pp_sc", bufs=2, space="PSUM"))
        pp_av = ctx.enter_context(tc.tile_pool(name="pp_av", bufs=2, space="PSUM"))

        # ---- constants ----
        ones_row = singles.tile([1, 128], F16, name="ones_row", tag="ones_row")
        nc.vector.memset(ones_row, 1.0)

        # (bias DMAs are emitted in the main schedule after the critical
        # cold-start tensors; they're only needed at first chain-end)
        sbq = singles.tile([128, NQT], F32, name="sbq", tag="sbq")
        sbk = singles.tile([128, 1], F32, name="sbk", tag="sbk")
        bv_row = singles.tile([1, KC], F16, name="bv_row", tag="bv_row")

        def emit_bias_dma():
            nc.sync.dma_start(out=sbq, in_=bq.rearrange("(t p) -> p t", p=128))
            nc.sync.dma_start(out=sbk, in_=bk.rearrange("(t p) -> p t", p=128))
            nc.sync.dma_start(
                out=bv_row, in_=bvh.rearrange("(a k) -> a k", a=1)
            )

        # ---- persistent weight tiles (fp8 hi/lo for QKV; wq one DMA per
        # (q-tile, half) so the first Q chain starts after 0.25MB) ----
        wq_qt = [
            wpool.tile([128, 2, NKD, 128], F8, name=f"wq_qt{qt}",
                       tag=f"wq{qt}")
            for qt in range(NQT)
        ]
        wk_all = wpool.tile([128, 2, NKD, KC], F8, name="wk_all",
                            tag="wk_all")
        wv_all = wpool.tile([128, 2, NKD, KC], F8, name="wv_all",
                            tag="wv_all")
        wo_all = wpool.tile([128, 2, NPR, D], F8, name="wo_all", tag="wo_all")

        def emit_wq_dma(qt):
            for h in range(2):
                nc.sync.dma_start(out=wq_qt[qt][:, h, :, :], in_=wqc[qt, h])

        def emit_wo_dma():
            for h in range(2):
                nc.sync.dma_start(out=wo_all[:, h, :, :], in_=woc[h])

        # ---- persistent K^T (per block) and V_aug ----
        kTb = []
        for j in range(NB):
            t = wpool.tile([128, SB], F16, name=f"kTb{j}", tag=f"kTb{j}")
            kTb.append(t)
        # vaug[g][j][:, tt, 0:64] = V rows for t-chunk (j*TPB+tt), group g;
        # col 64 = ones (folds the softmax denominator into the AV matmul).
        vaug = [[None] * NB for _ in range(GPC)]
        for g in range(GPC):
            for j in range(NB):
                t = wpool.tile(
                    [128, TPB, DK + 1], F16,
                    name=f"vaug{g}_{j}", tag=f"vaug{g}_{j}",
                )
                nc.vector.memset(t[:, :, DK:DK + 1], 1.0)
                vaug[g][j] = t

        xt_tiles = {}

        def emit_xt_dma(j):
            s0 = j * SB
            xt_all = xtp.tile([128, 2, NKD, SB], F8, name="xt_all",
                              tag="xt_all")
            for h in range(2):  # hi half first: hi*hi passes start earlier
                nc.sync.dma_start(
                    out=xt_all[:, h, :, :], in_=xTc[:, h, :, s0:s0 + SB]
                )
            xt_tiles[j] = xt_all

        # (x_half, w_half) per compensated pass: hi*hi + lo*hi + hi*lo
        PASSES = ((0, 0), (0, 1), (1, 0))

        qT_tiles = {}

        def emit_proj_parts(j):
            """Returns a list of closures, each emitting one projection
            chain for block j (4 Q tiles, 1 K tile, 4 V t-tiles)."""
            xt = xt_tiles[j]
            qT = [None] * NQT
            qT_tiles[j] = qT
            parts = []

            def q_part(qt):
                def emit():
                    ps = pp_pj.tile([128, SB], F32, name="ps_q", tag="pj")
                    for pi, (hx, hw) in enumerate(PASSES):
                        for kd in range(0, NKD, 2):
                            nc.tensor.matmul(
                                out=ps,
                                lhsT=wq_qt[qt][:, hw, kd:kd + 2, :],
                                rhs=xt[:, hx, kd:kd + 2, :],
                                start=(pi == 0 and kd == 0),
                                stop=(pi == 2 and kd == NKD - 2),
                                perf_mode=mybir.MatmulPerfMode.DoubleRow,
                            )
                    t = qtp.tile([128, SB], F16, name=f"qT{qt}", tag=f"qT{qt}")
                    nc.vector.tensor_scalar(
                        out=t, in0=ps, scalar1=ISCL,
                        scalar2=sbq[:, qt:qt + 1],
                        op0=mybir.AluOpType.mult, op1=mybir.AluOpType.add,
                    )
                    qT[qt] = t
                return emit

            def k_part():
                def emit():
                    ps = pp_pj.tile([128, SB], F32, name="ps_k", tag="pj")
                    for pi, (hx, hw) in enumerate(PASSES):
                        for kd in range(0, NKD, 2):
                            nc.tensor.matmul(
                                out=ps,
                                lhsT=wk_all[:, hw, kd:kd + 2, :],
                                rhs=xt[:, hx, kd:kd + 2, :],
                                start=(pi == 0 and kd == 0),
                                stop=(pi == 2 and kd == NKD - 2),
                                perf_mode=mybir.MatmulPerfMode.DoubleRow,
                            )
                    nc.vector.tensor_scalar(
                        out=kTb[j], in0=ps, scalar1=ISCL, scalar2=sbk,
                        op0=mybir.AluOpType.mult, op1=mybir.AluOpType.add,
                    )
                return emit

            def v_part(tt):
                def emit():
                    # V in natural [t, kv] layout: x chunk stationary.
                    # bv_row arrives pre-scaled by SCL from the host.
                    ps = pp_pj.tile([128, SB], F32, name="ps_v", tag="pj")
                    nc.tensor.matmul(
                        out=ps[:, 0:KC], lhsT=ones_row, rhs=bv_row,
                        start=True, stop=False,
                    )
                    for pi, (hx, hw) in enumerate(PASSES):
                        for kd in range(0, NKD, 2):
                            nc.tensor.matmul(
                                out=ps[:, 0:KC],
                                lhsT=xt[:, hx, kd:kd + 2,
                                        tt * 128:(tt + 1) * 128],
                                rhs=wv_all[:, hw, kd:kd + 2, :],
                                start=False,
                                stop=(pi == 2 and kd == NKD - 2),
                                perf_mode=mybir.MatmulPerfMode.DoubleRow,
                            )
                    for g in range(GPC):
                        nc.vector.tensor_scalar_mul(
                            out=vaug[g][j][:, tt, 0:DK],
                            in0=ps[:, g * DK:(g + 1) * DK],
                            scalar1=ISCL,
                        )
                return emit

            def kq0_part():
                # K and Q0 chains interleaved by kd (both pj banks): both
                # finish chasing the x DMA tail instead of serializing.
                def emit():
                    ps_k = pp_pj.tile([128, SB], F32, name="ps_k", tag="pj")
                    ps_q = pp_pj.tile([128, SB], F32, name="ps_q", tag="pj")
                    for pi, (hx, hw) in enumerate(PASSES):
                        for kd in range(0, NKD, 2):
                            st_ = (pi == 0 and kd == 0)
                            sp_ = (pi == 2 and kd == NKD - 2)
                            nc.tensor.matmul(
                                out=ps_k,
                                lhsT=wk_all[:, hw, kd:kd + 2, :],
                                rhs=xt[:, hx, kd:kd + 2, :],
                                start=st_, stop=sp_,
                                perf_mode=mybir.MatmulPerfMode.DoubleRow,
                            )
                            nc.tensor.matmul(
                                out=ps_q,
                                lhsT=wq_qt[0][:, hw, kd:kd + 2, :],
                                rhs=xt[:, hx, kd:kd + 2, :],
                                start=st_, stop=sp_,
                                perf_mode=mybir.MatmulPerfMode.DoubleRow,
                            )
                    nc.vector.tensor_scalar(
                        out=kTb[j], in0=ps_k, scalar1=ISCL, scalar2=sbk,
                        op0=mybir.AluOpType.mult, op1=mybir.AluOpType.add,
                    )
                    t = qtp.tile([128, SB], F16, name="qT0", tag="qT0")
                    nc.vector.tensor_scalar(
                        out=t, in0=ps_q, scalar1=ISCL, scalar2=sbq[:, 0:1],
                        op0=mybir.AluOpType.mult, op1=mybir.AluOpType.add,
                    )
                    qT[0] = t
                return emit

            if j == 0:
                # cold start: fused K||Q0; V parts go to pre-AV fillers
                parts.append(kq0_part())
            else:
                parts.append(k_part())
                parts.append(q_part(0))
            for tt in range(TPB):
                parts.append(v_part(tt))
            for qt in range(1, NQT):
                parts.append(q_part(qt))
            return parts

        apairs_by_block = {}

        def emit_oproj_parts(j, batch=4, act_copies=False):
            """Out-projection of block j: 16 column-tile closures. outT
            writes are DMA'd `batch` column-tiles at a time. With
            act_copies, alternate PSUM->SBUF copies between DVE and Act
            (for the final block, where Act is otherwise drained)."""
            s0 = j * SB
            apairs = apairs_by_block[j]

            osb4 = [None]

            def o_part(ot):
                def emit():
                    ps_o = pp_pj.tile([128, SB], F32, name="ps_o", tag="pj")
                    for pi, (hb, hw) in enumerate(PASSES):
                        for rp in range(NPR // 2):
                            nc.tensor.matmul(
                                out=ps_o,
                                lhsT=wo_all[:, hw, 2 * rp:2 * rp + 2,
                                            ot * 128:(ot + 1) * 128],
                                rhs=apairs[rp][:, :, :, hb],
                                start=(pi == 0 and rp == 0),
                                stop=(pi == 2 and rp == NPR // 2 - 1),
                                perf_mode=mybir.MatmulPerfMode.DoubleRow,
                            )
                    sub = ot % batch
                    if sub == 0:
                        osb4[0] = otp.tile([128, 4, SB], F16, name="osb",
                                           tag="osb")
                    if act_copies and ot % 2 == 0:
                        nc.scalar.activation(
                            out=osb4[0][:, sub, :], in_=ps_o,
                            func=mybir.ActivationFunctionType.Copy,
                            scale=ISCL,
                        )
                    else:
                        nc.vector.tensor_scalar_mul(
                            out=osb4[0][:, sub, :], in0=ps_o, scalar1=ISCL
                        )
                    if sub == batch - 1:
                        dst = outT[(ot - sub) * 128:(ot + 1) * 128,
                                   s0:s0 + SB]
                        nc.sync.dma_start(
                            out=dst.rearrange("(o p) s -> p o s", p=128),
                            in_=osb4[0][:, 0:batch, :],
                        )
                return emit

            return [o_part(ot) for ot in range(NOT)]

        def emit_attention(j, fillers, pre_av=()):
            """Attention for block j. `fillers` is a list of closures
            (PE-heavy, dependency-free work) drained into the stream to
            fill Act-bound stalls. `pre_av` closures are emitted after the
            first head's score chunks, before its AV groups (used for
            block 0's V parts, which that AV phase depends on)."""
            nti = TPB * (j + 1)
            qT = qT_tiles[j]
            # Spread fillers evenly over the block's fill slots (one slot
            # per (head, s-tile)) so later heads aren't starved.
            n_slots = HL * TPB
            fi = [0]
            slot = [0]

            def fill(last=False):
                slot[0] += 1
                want = len(fillers) if last else (
                    len(fillers) * slot[0] + n_slots - 1) // n_slots
                while fi[0] < min(want, len(fillers)):
                    fillers[fi[0]]()
                    fi[0] += 1

            # aflip/apairs carry fp8 hi/lo pairs packed in the two bytes of
            # each 16-bit lane (so one 2-byte DMA transpose moves both).
            aflip = [
                afp.tile([128, TPB, GPC, DK, 2], F8, name=f"af{r}",
                         tag=f"af{r}")
                for r in range(REP)
            ]
            apairs = [
                atp.tile([128, 2, SB, 2], F8, name=f"ap{rp}", tag=f"ap{rp}")
                for rp in range(NPR // 2)
            ]
            apairs_by_block[j] = apairs

            # Head order (g, r): g-major so that after head (1, r) both
            # group slices of aflip[r] are complete and can be transposed.
            for g in range(GPC):
                for r in range(REP):
                    qtile = qT[r]
                    qrow = g * DK
                    # Score chunks live in [128, 2, SB] pair tiles (2 PSUM
                    # banks); off-diagonal pairs share one 1024-col exp.
                    pair_pt = [None] * (nti // 2)
                    for ti in range(nti):
                        sub = ti % 2
                        if sub == 0:
                            ps_p = pp_sc.tile([128, 2, SB], F32, name="ps_p",
                                              tag="sc")
                            pt2 = ptp.tile([128, 2, SB], F16,
                                           name=f"pt{ti // 2}",
                                           tag=f"pt{ti // 2}")
                            pair_pt[ti // 2] = pt2
                        krel = ti - TPB * j
                        c0 = 128 * krel if krel > 0 else 0
                        nc.tensor.matmul(
                            out=ps_p[:, sub, c0:SB],
                            lhsT=kTb[ti // TPB][g * DK:(g + 1) * DK,
                                               (ti % TPB) * 128:
                                               (ti % TPB + 1) * 128],
                            rhs=qtile[qrow:qrow + DK, c0:SB],
                            start=True,
                            stop=True,
                        )
                        if krel >= 0:
                            nc.scalar.activation(
                                out=pt2[:, sub, c0:SB],
                                in_=ps_p[:, sub, c0:SB],
                                func=mybir.ActivationFunctionType.Exp,
                                scale=0.125,
                            )
                            # causal mask: zero the upper triangle of the
                            # diagonal 128x128 block post-exp, on Pool
                            # (keep where col - row >= 0, else fill 0)
                            nc.gpsimd.affine_select(
                                out=pt2[:, sub, c0:c0 + 128],
                                in_=pt2[:, sub, c0:c0 + 128],
                                compare_op=mybir.AluOpType.is_ge,
                                fill=0.0,
                                base=0,
                                pattern=[[1, 128]],
                                channel_multiplier=-1,
                            )
                        elif sub == 1:
                            # both chunks of an off-diagonal pair: one exp
                            nc.scalar.activation(
                                out=pt2[:, :, :], in_=ps_p[:, :, :],
                                func=mybir.ActivationFunctionType.Exp,
                                scale=0.125,
                            )

                    if g == 0 and r == 0:
                        for p in pre_av:
                            p()

                    # AV per s-tile: P stationary, V_aug moving (N=65).
                    for st in range(TPB):
                        jst = TPB * j + st
                        ps_av = pp_av.tile([128, DK + 1], F32, name="ps_av",
                                           tag="av")
                        for ti in range(jst + 1):
                            nc.tensor.matmul(
                                out=ps_av,
                                lhsT=pair_pt[ti // 2][:, ti % 2,
                                                      st * 128:(st + 1) * 128],
                                rhs=vaug[g][ti // TPB][:, ti % TPB, :],
                                start=(ti == 0),
                                stop=(ti == jst),
                            )
                        rcp = rcpool.tile([128, 1], F32, name="rcp", tag="rcp")
                        nc.vector.reciprocal(out=rcp, in_=ps_av[:, DK:DK + 1])
                        nc.vector.tensor_scalar_mul(
                            out=aflip[r][:, st, g, :, 0],
                            in0=ps_av[:, 0:DK],
                            scalar1=rcp,
                        )
                        # residual: lo = (av * rcp) - hi
                        nc.vector.scalar_tensor_tensor(
                            out=aflip[r][:, st, g, :, 1],
                            in0=ps_av[:, 0:DK],
                            scalar=rcp,
                            in1=aflip[r][:, st, g, :, 0],
                            op0=mybir.AluOpType.mult,
                            op1=mybir.AluOpType.subtract,
                        )
                        fill()
                    if g == 1:
                        # aflip[r] complete: [s, (g, dk)] -> [hd, s] via
                        # SBUF->SBUF DMA transpose (uint16 lanes = packed
                        # hi/lo fp8) into the pr-pair apair tile.
                        u16 = mybir.dt.uint16
                        for st in range(TPB):
                            nc.sync.dma_start_transpose(
                                out=apairs[r // 2].bitcast(u16)[
                                    :, r % 2, st * 128:(st + 1) * 128
                                ],
                                in_=aflip[r][:, st, :, :, :].bitcast(u16),
                            )
            fill(last=True)

        # ---- main schedule ----
        nc.sync.dma_start(out=wk_all[:, 0, :, :], in_=wkc[0])
        nc.sync.dma_start(out=wk_all[:, 1, :, :], in_=wkc[1])
        emit_xt_dma(0)
        emit_wq_dma(0)
        emit_bias_dma()
        nc.sync.dma_start(out=wv_all[:, 0, :, :], in_=wvc[0])
        nc.sync.dma_start(out=wv_all[:, 1, :, :], in_=wvc[1])
        for qt in range(1, NQT):
            emit_wq_dma(qt)
        a0 = emit_proj_parts(0)
        a0[0]()            # fused K||Q0; V0-3 run as h0 pre-AV fillers
        emit_xt_dma(1)
        emit_attention(0, a0[5:] + emit_proj_parts(1)
                       + [lambda: emit_xt_dma(2), emit_wo_dma],
                       pre_av=a0[1:5])
        c0 = emit_oproj_parts(0)
        emit_attention(1, emit_proj_parts(2) + [lambda: emit_xt_dma(3)]
                       + c0[:2])
        c1 = emit_oproj_parts(1)
        emit_attention(2, emit_proj_parts(3) + c0[2:10] + c1[:2])
        c2 = emit_oproj_parts(2)
        emit_attention(3, c0[10:] + c1[2:] + c2)
        for p in emit_oproj_parts(3, batch=2, act_copies=True):
            p()

    nc.compile()
    return nc


def make_in_maps(x, Wq, bq, Wk, bk, Wv, bv, Wo, bo):
    x = np.asarray(x, dtype=np.float32)
    Wq = np.asarray(Wq, dtype=np.float32)
    Wk = np.asarray(Wk, dtype=np.float32)
    Wv = np.asarray(Wv, dtype=np.float32)
    Wo = np.asarray(Wo, dtype=np.float32)
    bq = np.asarray(bq, dtype=np.float32)
    bk = np.asarray(bk, dtype=np.float32)
    bv = np.asarray(bv, dtype=np.float32)
    # Local-head layout permutation: q-tile m = [head m (g0) | head 4+m (g1)]
    perm = [0, REP, 1, REP + 1, 2, REP + 2, 3, REP + 3][:HL]
    in_maps = []
    for c in range(NCORES):
        b = c // (NCORES // B)
        gp = c % (NCORES // B)
        q0 = gp * QC
        k0 = gp * KC
        qcols = np.concatenate(
            [np.arange(q0 + hl * DK, q0 + (hl + 1) * DK) for hl in perm]
        )
        import ml_dtypes
        f8 = ml_dtypes.float8_e4m3

        def chunk_major(m, dtype=np.float16):
            # [R, C] -> [128, R//128, C]: out[p, kd, c] = m[kd*128+p, c]
            return np.ascontiguousarray(
                m.astype(dtype).reshape(-1, 128, m.shape[1]).transpose(1, 0, 2)
            )

        def split8(m, axis):
            # stack fp8 hi and residual lo along a new `axis`
            hi = m.astype(np.float32).astype(f8)
            lo = (m.astype(np.float32) - hi.astype(np.float32)).astype(f8)
            return np.ascontiguousarray(np.stack([hi, lo], axis=axis))

        wq_cm = chunk_major(Wq[:, qcols] * 32.0, np.float32)  # [128,NKD,QC]
        wq_qt_maj = wq_cm.reshape(128, NKD, NQT, 128).transpose(2, 0, 1, 3)
        in_maps.append({
            "xTc": split8(chunk_major(x[b].T, np.float32), axis=1),
            "wqc": split8(wq_qt_maj, axis=1),
            "wkc": split8(chunk_major(Wk[:, k0:k0 + KC] * 32.0, np.float32),
                          axis=0),
            "wvc": split8(chunk_major(Wv[:, k0:k0 + KC] * 32.0, np.float32),
                          axis=0),
            "woc": split8(chunk_major(Wo[qcols, :] * 32.0, np.float32),
                          axis=0),
            "bq": np.ascontiguousarray(bq[qcols]),
            "bk": np.ascontiguousarray(bk[k0:k0 + KC]),
            "bvh": np.ascontiguousarray(
                (bv[k0:k0 + KC] * 32.0).astype(np.float16)
            ),
        })
    return in_maps


def assemble_output(results, bo):
    bo = np.asarray(bo, dtype=np.float32)
    out = np.zeros((B, S, D), dtype=np.float32)
    for c in range(NCORES):
        b = c // (NCORES // B)
        out[b] += results[c]["outT"].T.astype(np.float32)
    out += bo
    return out


_NC_CACHE = None


def kernel(x, Wq, bq, Wk, bk, Wv, bv, Wo, bo):
    global _NC_CACHE
    from concourse.bass_utils import run_bass_kernel_spmd

    if _NC_CACHE is None:
        _NC_CACHE = build_gqa_nc()
    nc = _NC_CACHE
    in_maps = make_in_maps(x, Wq, bq, Wk, bk, Wv, bv, Wo, bo)
    res = run_bass_kernel_spmd(nc, in_maps, list(range(NCORES))).results
    return assemble_output(res, bo)


# revision 5
# speedup vs baseline: 1.1487x; 1.0044x over previous
"""Grouped-Query Attention kernel for Trainium2, 8-core SPMD. v2.

Problem (full shapes): B=2, S=2048, D=2048, H=32 q-heads, KV=8 kv-heads,
DK=64, REP=4.

Sharding: 16 (batch, kv-group) units over 8 cores -> each core owns one
batch b and 2 adjacent kv-groups (8 query heads, 512 q-cols / 128 kv-cols).
Each core computes its heads' attention output and a partial output
projection against its 512-row slice of Wo; the host sums the 4 partials
per batch and adds bo.

v2/v3 restructuring vs v1 (418.7us -> 224.6us on the cost model):
- Q/K/V and output projections run as fp8e4m3 DoubleRow matmuls (2
  K-chunks per instruction at 0.5 cyc/row = 4x f16 throughput) with
  hi/lo error compensation: W*32 split into fp8 hi + residual lo on
  the host, three passes (hi*hi + lo*hi + hi*lo) accumulate in PSUM,
  and the DVE epilogue rescales by 1/32. Attention output hi/lo fp8
  bytes are packed into uint16 lanes so one 2-byte DMA transpose
  carries both halves; the out-proj reads them as stride-2 fp8 APs.
  Scores/AV stay f16 (softmax is error-sensitive; K=64 contraction
  cannot use DoubleRow anyway). rel_err 7e-4 -> 6.5e-3, gate 2e-2.
- AV matmul flipped: out[s, dk+1] = P^T.T @ V_aug with the P tile as
  stationary (N=65 per chunk instead of 512) - halves AV PE cycles and
  puts the softmax denominator on the partition axis, so normalization
  is one reciprocal + one tensor_scalar_mul on DVE.
- Causal mask applied post-exp on the (otherwise idle) Pool engine:
  affine_select zeroes the upper triangle of the diagonal 128x128
  block of the f16 P tile in SBUF - no PE/DVE/Act mask work.
- Off-diagonal score chunks live in [128, 2, 512] 2-bank PSUM pair
  tiles; each pair shares one 1024-col exp (amortizes Act init).
- Attention output transposed back to [hd, s] with SBUF->SBUF DMA
  transposes (14 ns per 16x128 xbar tile) instead of PE transposes.
- V projected directly in [t, kv] layout (x chunk as stationary), bias
  folded in via a K=1 ones matmul - no V transposes.
- QKV bias adds on DVE (tensor_scalar_add), Act engine runs exps only.
- Software-pipelined emission: out-proj of block j-1 and QKV proj of
  block j+1 are interleaved (evenly spread) into the Act-bound
  attention phase of block j to keep the in-order PE stream fed;
  extra out-proj tiles are donated to the most Act-bound block 3.
- Inputs in chunk-major host layouts so each tensor loads as one big
  DMA (per-DMA fixed cost ~1.3us); block-0 x split in two halves with
  the K and Q0 chains interleaved across both pj PSUM banks so the
  cold start chases the DMA tail; outT written 4 column-tiles per DMA.
- f16 output partials (halves output DMA; host sums in f32).
"""

import os
from contextlib import ExitStack

import numpy as np

import concourse.bass as bass
import concourse.tile as tile
from concourse import bacc
from concourse import mybir

F32 = mybir.dt.float32
F16 = mybir.dt.float16
F8 = mybir.dt.float8e4

# fp8 hi/lo compensated projections: W pre-scaled by SCL on the host so
# W*SCL sits in e4m3's normal range; the DVE epilogue multiplies by 1/SCL.
SCL = 32.0
ISCL = 1.0 / SCL

# Full-problem constants (hardcoded per contest contract).
B = 2
S = 2048
D = 2048
H = 32
KV = 8
DK = 64
REP = H // KV          # 4
NCORES = 8

GPC = (KV * B) // NCORES      # kv-groups per core = 2
QC = GPC * REP * DK           # local q cols = 512
KC = GPC * DK                 # local k cols = 128
HL = GPC * REP                # local heads = 8
SB = 512                      # s-block size
NB = S // SB                  # 4 blocks
NKD = D // 128                # 16 contraction chunks for projections
NQT = QC // 128               # 4 q-col tiles
NPR = QC // 128               # 4 head-pair tiles (rhs chunks for out proj)
NOT = D // 128                # 16 out-col tiles
TPB = SB // 128               # 4 t-chunks per s-block



def build_gqa_nc():
    nc = bacc.Bacc("TRN2", target_bir_lowering=False, debug=False)

    # Chunk-major layouts (prepared on host): tensor[p, kd, ...] holds row
    # kd*128+p of the logical matrix, so each loads as ONE big DMA with
    # multi-KB contiguous runs per partition (per-DMA fixed costs are
    # ~1.3 us; 128KB-tile loads would pay ~60% overhead).
    # fp8 hi/lo pairs (dim of size 2 = hi/lo) for the QKV projections;
    # total bytes match the old f16 layouts.
    xTc = nc.dram_tensor(
        "xTc", [128, 2, NKD, S], F8, kind="ExternalInput"
    ).ap()
    wqc = nc.dram_tensor(
        "wqc", [NQT, 2, 128, NKD, 128], F8, kind="ExternalInput"
    ).ap()
    wkc = nc.dram_tensor(
        "wkc", [2, 128, NKD, KC], F8, kind="ExternalInput"
    ).ap()
    wvc = nc.dram_tensor(
        "wvc", [2, 128, NKD, KC], F8, kind="ExternalInput"
    ).ap()
    woc = nc.dram_tensor(
        "woc", [2, 128, NPR, D], F8, kind="ExternalInput"
    ).ap()
    bq = nc.dram_tensor("bq", [QC], F32, kind="ExternalInput").ap()
    bk = nc.dram_tensor("bk", [KC], F32, kind="ExternalInput").ap()
    bvh = nc.dram_tensor("bvh", [KC], F16, kind="ExternalInput").ap()
    outT = nc.dram_tensor("outT", [D, S], F16, kind="ExternalOutput").ap()

    with tile.TileContext(nc) as tc, ExitStack() as ctx:
        singles = ctx.enter_context(tc.tile_pool(name="singles", bufs=1))
        wpool = ctx.enter_context(tc.tile_pool(name="wpool", bufs=1))
        xtp = ctx.enter_context(tc.tile_pool(name="xtp", bufs=2))
        qtp = ctx.enter_context(tc.tile_pool(name="qtp", bufs=2))
        ptp = ctx.enter_context(tc.tile_pool(name="ptp", bufs=2))
        afp = ctx.enter_context(tc.tile_pool(name="afp", bufs=2))
        atp = ctx.enter_context(tc.tile_pool(name="atp", bufs=4))
        otp = ctx.enter_context(tc.tile_pool(name="otp", bufs=3))
        rcpool = ctx.enter_context(tc.tile_pool(name="rcpool", bufs=8))

        pp_pj = ctx.enter_context(tc.tile_pool(name="pp_pj", bufs=2, space="PSUM"))
        pp_sc = ctx.enter_context(tc.tile_pool(name="pp_sc", bufs=2, space="PSUM"))
        pp_av = ctx.enter_context(tc.tile_pool(name="pp_av", bufs=2, space="PSUM"))

        # ---- constants ----
        ones_row = singles.tile([1, 128], F16, name="ones_row", tag="ones_row")
        nc.vector.memset(ones_row, 1.0)

        # (bias DMAs are emitted in the main schedule after the critical
        # cold-start tensors; they're only needed at first chain-end)
        sbq = singles.tile([128, NQT], F32, name="sbq", tag="sbq")
        sbk = singles.tile([128, 1], F32, name="sbk", tag="sbk")
        bv_row = singles.tile([1, KC], F16, name="bv_row", tag="bv_row")

        def emit_bias_dma():
            nc.sync.dma_start(out=sbq, in_=bq.rearrange("(t p) -> p t", p=128))
            nc.sync.dma_start(out=sbk, in_=bk.rearrange("(t p) -> p t", p=128))
            nc.sync.dma_start(
                out=bv_row, in_=bvh.rearrange("(a k) -> a k", a=1)
            )

        # ---- persistent weight tiles (fp8 hi/lo for QKV; wq one DMA per
        # (q-tile, half) so the first Q chain starts after 0.25MB) ----
        wq_qt = [
            wpool.tile([128, 2, NKD, 128], F8, name=f"wq_qt{qt}",
                       tag=f"wq{qt}")
            for qt in range(NQT)
        ]
        wk_all = wpool.tile([128, 2, NKD, KC], F8, name="wk_all",
                            tag="wk_all")
        wv_all = wpool.tile([128, 2, NKD, KC], F8, name="wv_all",
                            tag="wv_all")
        wo_all = wpool.tile([128, 2, NPR, D], F8, name="wo_all", tag="wo_all")

        def emit_wq_dma(qt):
            for h in range(2):
                nc.sync.dma_start(out=wq_qt[qt][:, h, :, :], in_=wqc[qt, h])

        def emit_wo_dma():
            for h in range(2):
                nc.sync.dma_start(out=wo_all[:, h, :, :], in_=woc[h])

        # ---- persistent K^T (per block) and V_aug ----
        kTb = []
        for j in range(NB):
            t = wpool.tile([128, SB], F16, name=f"kTb{j}", tag=f"kTb{j}")
            kTb.append(t)
        # vaug[g][j][:, tt, 0:64] = V rows for t-chunk (j*TPB+tt), group g;
        # col 64 = ones (folds the softmax denominator into the AV matmul).
        vaug = [[None] * NB for _ in range(GPC)]
        for g in range(GPC):
            for j in range(NB):
                t = wpool.tile(
                    [128, TPB, DK + 1], F16,
                    name=f"vaug{g}_{j}", tag=f"vaug{g}_{j}",
                )
                nc.vector.memset(t[:, :, DK:DK + 1], 1.0)
                vaug[g][j] = t

        xt_tiles = {}

        def emit_xt_dma(j):
            s0 = j * SB
            xt_all = xtp.tile([128, 2, NKD, SB], F8, name="xt_all",
                              tag="xt_all")
            for h in range(2):  # hi half first: hi*hi passes start earlier
                nc.sync.dma_start(
                    out=xt_all[:, h, :, :], in_=xTc[:, h, :, s0:s0 + SB]
                )
            xt_tiles[j] = xt_all

        # (x_half, w_half) per compensated pass: hi*hi + lo*hi + hi*lo
        PASSES = ((0, 0), (0, 1), (1, 0))

        qT_tiles = {}

        def emit_proj_parts(j):
            """Returns a list of closures, each emitting one projection
            chain for block j (4 Q tiles, 1 K tile, 4 V t-tiles)."""
            xt = xt_tiles[j]
            qT = [None] * NQT
            qT_tiles[j] = qT
            parts = []

            def q_part(qt):
                def emit():
                    ps = pp_pj.tile([128, SB], F32, name="ps_q", tag="pj")
                    for pi, (hx, hw) in enumerate(PASSES):
                        for kd in range(0, NKD, 2):
                            nc.tensor.matmul(
                                out=ps,
                                lhsT=wq_qt[qt][:, hw, kd:kd + 2, :],
                                rhs=xt[:, hx, kd:kd + 2, :],
                                start=(pi == 0 and kd == 0),
                                stop=(pi == 2 and kd == NKD - 2),
                                perf_mode=mybir.MatmulPerfMode.DoubleRow,
                            )
                    t = qtp.tile([128, SB], F16, name=f"qT{qt}", tag=f"qT{qt}")
                    nc.vector.tensor_scalar(
                        out=t, in0=ps, scalar1=ISCL,
                        scalar2=sbq[:, qt:qt + 1],
                        op0=mybir.AluOpType.mult, op1=mybir.AluOpType.add,
                    )
                    qT[qt] = t
                return emit

            def k_part():
                def emit():
                    ps = pp_pj.tile([128, SB], F32, name="ps_k", tag="pj")
                    for pi, (hx, hw) in enumerate(PASSES):
                        for kd in range(0, NKD, 2):
                            nc.tensor.matmul(
                                out=ps,
                                lhsT=wk_all[:, hw, kd:kd + 2, :],
                                rhs=xt[:, hx, kd:kd + 2, :],
                                start=(pi == 0 and kd == 0),
                                stop=(pi == 2 and kd == NKD - 2),
                                perf_mode=mybir.MatmulPerfMode.DoubleRow,
                            )
                    nc.vector.tensor_scalar(
                        out=kTb[j], in0=ps, scalar1=ISCL, scalar2=sbk,
                        op0=mybir.AluOpType.mult, op1=mybir.AluOpType.add,
                    )
                return emit

            def v_part(tt):
                def emit():
                    # V in natural [t, kv] layout: x chunk stationary.
                    # bv_row arrives pre-scaled by SCL from the host.
                    ps = pp_pj.tile([128, SB], F32, name="ps_v", tag="pj")
                    nc.tensor.matmul(
                        out=ps[:, 0:KC], lhsT=ones_row, rhs=bv_row,
                        start=True, stop=False,
                    )
                    for pi, (hx, hw) in enumerate(PASSES):
                        for kd in range(0, NKD, 2):
                            nc.tensor.matmul(
                                out=ps[:, 0:KC],
                                lhsT=xt[:, hx, kd:kd + 2,
                                        tt * 128:(tt + 1) * 128],
                                rhs=wv_all[:, hw, kd:kd + 2, :],
                                start=False,
                                stop=(pi == 2 and kd == NKD - 2),
                                perf_mode=mybir.MatmulPerfMode.DoubleRow,
                            )
                    for g in range(GPC):
                        nc.vector.tensor_scalar_mul(
                            out=vaug[g][j][:, tt, 0:DK],
                            in0=ps[:, g * DK:(g + 1) * DK],
                            scalar1=ISCL,
                        )
                return emit

            def kq0_part():
                # K and Q0 chains interleaved by kd (both pj banks): both
                # finish chasing the x DMA tail instead of serializing.
                def emit():
                    ps_k = pp_pj.tile([128, SB], F32, name="ps_k", tag="pj")
                    ps_q = pp_pj.tile([128, SB], F32, name="ps_q", tag="pj")
                    for pi, (hx, hw) in enumerate(PASSES):
                        for kd in range(0, NKD, 2):
                            st_ = (pi == 0 and kd == 0)
                            sp_ = (pi == 2 and kd == NKD - 2)
                            nc.tensor.matmul(
                                out=ps_k,
                                lhsT=wk_all[:, hw, kd:kd + 2, :],
                                rhs=xt[:, hx, kd:kd + 2, :],
                                start=st_, stop=sp_,
                                perf_mode=mybir.MatmulPerfMode.DoubleRow,
                            )
                            nc.tensor.matmul(
                                out=ps_q,
                                lhsT=wq_qt[0][:, hw, kd:kd + 2, :],
                                rhs=xt[:, hx, kd:kd + 2, :],
                                start=st_, stop=sp_,
                                perf_mode=mybir.MatmulPerfMode.DoubleRow,
                            )
                    nc.vector.tensor_scalar(
                        out=kTb[j], in0=ps_k, scalar1=ISCL, scalar2=sbk,
                        op0=mybir.AluOpType.mult, op1=mybir.AluOpType.add,
                    )
                    t = qtp.tile([128, SB], F16, name="qT0", tag="qT0")
                    nc.vector.tensor_scalar(
                        out=t, in0=ps_q, scalar1=ISCL, scalar2=sbq[:, 0:1],
                        op0=mybir.AluOpType.mult, op1=mybir.AluOpType.add,
                    )
                    qT[0] = t
                return emit

            if j == 0:
                # cold start: fused K||Q0; V parts go to pre-AV fillers
                parts.append(kq0_part())
            else:
                parts.append(k_part())
                parts.append(q_part(0))
            for tt in range(TPB):
                parts.append(v_part(tt))
            for qt in range(1, NQT):
                parts.append(q_part(qt))
            return parts

        apairs_by_block = {}

        def emit_oproj_parts(j, batch=4, act_copies=False):
            """Out-projection of block j: 16 column-tile closures. outT
            writes are DMA'd `batch` column-tiles at a time. With
            act_copies, alternate PSUM->SBUF copies between DVE and Act
            (for the final block, where Act is otherwise drained)."""
            s0 = j * SB
            apairs = apairs_by_block[j]

            osb4 = [None]

            def o_part(ot):
                def emit():
                    ps_o = pp_pj.tile([128, SB], F32, name="ps_o", tag="pj")
                    for pi, (hb, hw) in enumerate(PASSES):
                        for rp in range(NPR // 2):
                            nc.tensor.matmul(
                                out=ps_o,
                                lhsT=wo_all[:, hw, 2 * rp:2 * rp + 2,
                                            ot * 128:(ot + 1) * 128],
                                rhs=apairs[rp][:, :, :, hb],
                                start=(pi == 0 and rp == 0),
                                stop=(pi == 2 and rp == NPR // 2 - 1),
                                perf_mode=mybir.MatmulPerfMode.DoubleRow,
                            )
                    sub = ot % batch
                    if sub == 0:
                        osb4[0] = otp.tile([128, 4, SB], F16, name="osb",
                                           tag="osb")
                    if act_copies and ot % 2 == 0:
                        nc.scalar.activation(
                            out=osb4[0][:, sub, :], in_=ps_o,
                            func=mybir.ActivationFunctionType.Copy,
                            scale=ISCL,
                        )
                    else:
                        nc.vector.tensor_scalar_mul(
                            out=osb4[0][:, sub, :], in0=ps_o, scalar1=ISCL
                        )
                    if sub == batch - 1:
                        dst = outT[(ot - sub) * 128:(ot + 1) * 128,
                                   s0:s0 + SB]
                        nc.sync.dma_start(
                            out=dst.rearrange("(o p) s -> p o s", p=128),
                            in_=osb4[0][:, 0:batch, :],
                        )
                return emit

            return [o_part(ot) for ot in range(NOT)]

        def emit_attention(j, fillers, pre_av=()):
            """Attention for block j. `fillers` is a list of closures
            (PE-heavy, dependency-free work) drained into the stream to
            fill Act-bound stalls. `pre_av` closures are emitted after the
            first head's score chunks, before its AV groups (used for
            block 0's V parts, which that AV phase depends on)."""
            nti = TPB * (j + 1)
            qT = qT_tiles[j]
            # Spread fillers evenly over the block's fill slots (one slot
            # per (head, s-tile)) so later heads aren't starved.
            n_slots = HL * TPB
            fi = [0]
            slot = [0]

            def fill(last=False):
                slot[0] += 1
                want = len(fillers) if last else (
                    len(fillers) * slot[0] + n_slots - 1) // n_slots
                while fi[0] < min(want, len(fillers)):
                    fillers[fi[0]]()
                    fi[0] += 1

            # aflip/apairs carry fp8 hi/lo pairs packed in the two bytes of
            # each 16-bit lane (so one 2-byte DMA transpose moves both).
            aflip = [
                afp.tile([128, TPB, GPC, DK, 2], F8, name=f"af{r}",
                         tag=f"af{r}")
                for r in range(REP)
            ]
            apairs = [
                atp.tile([128, 2, SB, 2], F8, name=f"ap{rp}", tag=f"ap{rp}")
                for rp in range(NPR // 2)
            ]
            apairs_by_block[j] = apairs

            # Head order (g, r): g-major so that after head (1, r) both
            # group slices of aflip[r] are complete and can be transposed.
            for g in range(GPC):
                for r in range(REP):
                    qtile = qT[r]
                    qrow = g * DK
                    # Score chunks live in [128, 2, SB] pair tiles (2 PSUM
                    # banks); off-diagonal pairs share one 1024-col exp.
                    # The first diagonal pair swaps sub-slots (krel0->sub1,
                    # krel1->sub0) so its two valid regions are contiguous
                    # in the flat view and also share one exp.
                    pair_pt = [None] * (nti // 2)
                    sub_of = [None] * nti

                    def diag_mask(pt2, sub, c0):
                        # zero the upper triangle of the diagonal 128x128
                        # block post-exp, on Pool (keep where col-row >= 0)
                        nc.gpsimd.affine_select(
                            out=pt2[:, sub, c0:c0 + 128],
                            in_=pt2[:, sub, c0:c0 + 128],
                            compare_op=mybir.AluOpType.is_ge,
                            fill=0.0,
                            base=0,
                            pattern=[[1, 128]],
                            channel_multiplier=-1,
                        )

                    for ti in range(nti):
                        krel = ti - TPB * j
                        sub = ti % 2
                        sub_of[ti] = sub
                        if ti % 2 == 0:
                            ps_p = pp_sc.tile([128, 2, SB], F32, name="ps_p",
                                              tag="sc")
                            pt2 = ptp.tile([128, 2, SB], F16,
                                           name=f"pt{ti // 2}",
                                           tag=f"pt{ti // 2}")
                            pair_pt[ti // 2] = pt2
                        c0 = 128 * krel if krel > 0 else 0
                        nc.tensor.matmul(
                            out=ps_p[:, sub, c0:SB],
                            lhsT=kTb[ti // TPB][g * DK:(g + 1) * DK,
                                               (ti % TPB) * 128:
                                               (ti % TPB + 1) * 128],
                            rhs=qtile[qrow:qrow + DK, c0:SB],
                            start=True,
                            stop=True,
                        )
                        if krel >= 0:
                            nc.scalar.activation(
                                out=pt2[:, sub, c0:SB],
                                in_=ps_p[:, sub, c0:SB],
                                func=mybir.ActivationFunctionType.Exp,
                                scale=0.125,
                            )
                            diag_mask(pt2, sub, c0)
                        elif krel < 0 and ti % 2 == 1:
                            # both chunks of an off-diagonal pair: one exp
                            nc.scalar.activation(
                                out=pt2[:, :, :], in_=ps_p[:, :, :],
                                func=mybir.ActivationFunctionType.Exp,
                                scale=0.125,
                            )

                    if g == 0 and r == 0:
                        for p in pre_av:
                            p()

                    # AV per s-tile: P stationary, V_aug moving (N=65).
                    for st in range(TPB):
                        jst = TPB * j + st
                        ps_av = pp_av.tile([128, DK + 1], F32, name="ps_av",
                                           tag="av")
                        for ti in range(jst + 1):
                            nc.tensor.matmul(
                                out=ps_av,
                                lhsT=pair_pt[ti // 2][:, sub_of[ti],
                                                      st * 128:(st + 1) * 128],
                                rhs=vaug[g][ti // TPB][:, ti % TPB, :],
                                start=(ti == 0),
                                stop=(ti == jst),
                            )
                        rcp = rcpool.tile([128, 1], F32, name="rcp", tag="rcp")
                        nc.vector.reciprocal(out=rcp, in_=ps_av[:, DK:DK + 1])
                        # one DVE op drains the av PSUM bank (frees it for
                        # the st+2 group); hi/lo split runs on idle Pool.
                        t32 = rcpool.tile([128, DK], F32, name="t32",
                                          tag="t32", bufs=6)
                        nc.vector.tensor_scalar_mul(
                            out=t32, in0=ps_av[:, 0:DK], scalar1=rcp
                        )
                        nc.gpsimd.tensor_copy(
                            out=aflip[r][:, st, g, :, 0], in_=t32
                        )
                        # residual: lo = (av * rcp) - hi
                        nc.gpsimd.tensor_sub(
                            out=aflip[r][:, st, g, :, 1],
                            in0=t32,
                            in1=aflip[r][:, st, g, :, 0],
                        )
                        fill()
                    if g == 1:
                        # aflip[r] complete: [s, (g, dk)] -> [hd, s] via
                        # SBUF->SBUF DMA transpose (uint16 lanes = packed
                        # hi/lo fp8) into the pr-pair apair tile.
                        u16 = mybir.dt.uint16
                        for st in range(TPB):
                            nc.sync.dma_start_transpose(
                                out=apairs[r // 2].bitcast(u16)[
                                    :, r % 2, st * 128:(st + 1) * 128
                                ],
                                in_=aflip[r][:, st, :, :, :].bitcast(u16),
                            )
            fill(last=True)

        # ---- main schedule ----
        nc.sync.dma_start(out=wk_all[:, 0, :, :], in_=wkc[0])
        nc.sync.dma_start(out=wk_all[:, 1, :, :], in_=wkc[1])
        emit_xt_dma(0)
        emit_wq_dma(0)
        emit_bias_dma()
        nc.sync.dma_start(out=wv_all[:, 0, :, :], in_=wvc[0])
        nc.sync.dma_start(out=wv_all[:, 1, :, :], in_=wvc[1])
        for qt in range(1, NQT):
            emit_wq_dma(qt)
        a0 = emit_proj_parts(0)
        a0[0]()            # fused K||Q0; V0-3 run as h0 pre-AV fillers
        emit_xt_dma(1)
        emit_attention(0, a0[5:] + emit_proj_parts(1)
                       + [lambda: emit_xt_dma(2), emit_wo_dma],
                       pre_av=a0[1:5])
        c0 = emit_oproj_parts(0)
        emit_attention(1, emit_proj_parts(2) + [lambda: emit_xt_dma(3)]
                       + c0[:2])
        c1 = emit_oproj_parts(1)
        emit_attention(2, emit_proj_parts(3) + c0[2:10] + c1[:2])
        c2 = emit_oproj_parts(2)
        emit_attention(3, c0[10:] + c1[2:] + c2)
        for p in emit_oproj_parts(3, batch=2, act_copies=True):
            p()

    nc.compile()
    return nc


def make_in_maps(x, Wq, bq, Wk, bk, Wv, bv, Wo, bo):
    x = np.asarray(x, dtype=np.float32)
    Wq = np.asarray(Wq, dtype=np.float32)
    Wk = np.asarray(Wk, dtype=np.float32)
    Wv = np.asarray(Wv, dtype=np.float32)
    Wo = np.asarray(Wo, dtype=np.float32)
    bq = np.asarray(bq, dtype=np.float32)
    bk = np.asarray(bk, dtype=np.float32)
    bv = np.asarray(bv, dtype=np.float32)
    # Local-head layout permutation: q-tile m = [head m (g0) | head 4+m (g1)]
    perm = [0, REP, 1, REP + 1, 2, REP + 2, 3, REP + 3][:HL]
    in_maps = []
    for c in range(NCORES):
        b = c // (NCORES // B)
        gp = c % (NCORES // B)
        q0 = gp * QC
        k0 = gp * KC
        qcols = np.concatenate(
            [np.arange(q0 + hl * DK, q0 + (hl + 1) * DK) for hl in perm]
        )
        import ml_dtypes
        f8 = ml_dtypes.float8_e4m3

        def chunk_major(m, dtype=np.float16):
            # [R, C] -> [128, R//128, C]: out[p, kd, c] = m[kd*128+p, c]
            return np.ascontiguousarray(
                m.astype(dtype).reshape(-1, 128, m.shape[1]).transpose(1, 0, 2)
            )

        def split8(m, axis):
            # stack fp8 hi and residual lo along a new `axis`
            hi = m.astype(np.float32).astype(f8)
            lo = (m.astype(np.float32) - hi.astype(np.float32)).astype(f8)
            return np.ascontiguousarray(np.stack([hi, lo], axis=axis))

        wq_cm = chunk_major(Wq[:, qcols] * 32.0, np.float32)  # [128,NKD,QC]
        wq_qt_maj = wq_cm.reshape(128, NKD, NQT, 128).transpose(2, 0, 1, 3)
        in_maps.append({
            "xTc": split8(chunk_major(x[b].T, np.float32), axis=1),
            "wqc": split8(wq_qt_maj, axis=1),
            "wkc": split8(chunk_major(Wk[:, k0:k0 + KC] * 32.0, np.float32),
                          axis=0),
            "wvc": split8(chunk_major(Wv[:, k0:k0 + KC] * 32.0, np.float32),
                          axis=0),
            "woc": split8(chunk_major(Wo[qcols, :] * 32.0, np.float32),
                          axis=0),
            "bq": np.ascontiguousarray(bq[qcols]),
            "bk": np.ascontiguousarray(bk[k0:k0 + KC]),
            "bvh": np.ascontiguousarray(
                (bv[k0:k0 + KC] * 32.0).astype(np.float16)
            ),
        })
    return in_maps


def assemble_output(results, bo):
    bo = np.asarray(bo, dtype=np.float32)
    out = np.zeros((B, S, D), dtype=np.float32)
    for c in range(NCORES):
        b = c // (NCORES // B)
        out[b] += results[c]["outT"].T.astype(np.float32)
    out += bo
    return out


_NC_CACHE = None


def kernel(x, Wq, bq, Wk, bk, Wv, bv, Wo, bo):
    global _NC_CACHE
    from concourse.bass_utils import run_bass_kernel_spmd

    if _NC_CACHE is None:
        _NC_CACHE = build_gqa_nc()
    nc = _NC_CACHE
    in_maps = make_in_maps(x, Wq, bq, Wk, bk, Wv, bv, Wo, bo)
    res = run_bass_kernel_spmd(nc, in_maps, list(range(NCORES))).results
    return assemble_output(res, bo)
